# revision 1
# baseline (speedup 1.0000x reference)
"""CRF forward-algorithm kernel for Trainium2 (8 NeuronCores, Bass).

Strategy: data-parallel over batch (32 -> 4 per core) PLUS chunked-scan
parallelism over time. The recursion

    alpha_t[b,j] = scores[b,t,j] + lse_i(trans[i,j] + alpha_{t-1}[b,i])

is run in linear space with a global per-step normalizer K:

    p_t[j,(c,b)] = exp(scores - K) * sum_i E[i,j] p_{t-1}[i,(c,b)]

The key observation: E = exp(trans) has entries in [e^-0.1, e^0.1], so the
linear map contracts the Hilbert projective metric by ~0.1 per step. T=512
is split into C=28 chunks of L=18 steps; every chunk runs concurrently as
extra matmul columns, warm-started W=8 steps early from a surrogate init
(alpha ~ emission scores). After W steps the state DIRECTION matches the
true chain to ~1e-8; only a per-chunk scalar log-offset delta remains,
recovered by a sequential stitch (prefix-sum over chunk boundary
mismatches, done as one small triangular matmul) and added to the output.
Sequential scan length drops 512 -> S = L+W = 26 steps.

Per step: one PE matmul (E stationary bf16, 112 moving columns) + one DVE
multiply. ln/transposes/Kt-correction pipeline behind the scan on ACT/PE/
DVE; the output leaves in a [(chunk,batch) partition, (step,tag)] layout
where the delta correction is a native per-partition tensor_scalar add,
and K*t - 10000*[j==0] is a host-precomputed constant fused into the
PSUM->SBUF copy. Host does only layout permutes (gather/scatter), no math.
"""

import numpy as np

N = 64
T = 512
B = 32
NCORES = 8
BS = B // NCORES   # 4 batch elements per core
C = 28             # time chunks
W = 8              # warmup steps per chunk
L = (T - W) // C   # 18 real steps per chunk (chunk 0: L+W)
S = L + W          # 26 sequential scan steps
CB = C * BS        # 112 columns per scan step
NCOL = S * CB      # 2912 state columns
K = 4.66
# es cols 0:112 ride in the consts DMA; the rest arrive in 3 DMA pieces
DCUM = [112, 1045, 1978, NCOL]        # cumulative cols per DMA completion
XCUM = [112, 448, 1045, 1978, NCOL]   # cumulative cols per exp op
FPIECE = 2         # final delta-add/DMA-out pieces
FC = S * N // FPIECE  # 832 cols per piece


def _piece(r):
    # exp ops needed before scan column-slice r is read
    need = (r + 1) * CB
    for i, c in enumerate(XCUM):
        if c >= need:
            return i + 1
    return len(XCUM)


def _build_program():
    import concourse.bass as bass
    import concourse.mybir as mybir

    FT = mybir.dt.float32
    BF = mybir.dt.bfloat16
    AF = mybir.ActivationFunctionType

    nc = bass.Bass()
    scp_d = nc.declare_dram_parameter("scp", [N, NCOL], FT, isOutput=False)
    cst_d = nc.declare_dram_parameter("consts", [CB, 2 * N + 3 + CB + CB], FT,
                                      isOutput=False)
    ktf_d = nc.declare_dram_parameter("ktfull", [CB, S * N], FT, isOutput=False)
    out_d = nc.declare_dram_parameter("out", [CB, S * N], FT, isOutput=True)

    from contextlib import ExitStack

    with ExitStack() as ctx:
        es_sc = ctx.enter_context(nc.sbuf_tensor([N, NCOL], FT))
        es = ctx.enter_context(nc.sbuf_tensor([N, NCOL], FT))
        p_all = ctx.enter_context(nc.sbuf_tensor([N, NCOL], BF))
        lnp = ctx.enter_context(nc.sbuf_tensor([N, NCOL], FT))
        e_sb = ctx.enter_context(nc.sbuf_tensor([N, N], BF))
        cst = ctx.enter_context(nc.sbuf_tensor([CB, 2 * N + 3 + CB + CB], FT))
        scr = ctx.enter_context(nc.sbuf_tensor([N, 1], FT))
        e0k = ctx.enter_context(nc.sbuf_tensor([N, 1], FT))
        tr_nat = cst[0:N, 0:N]
        tcol_sb = cst[0:N, N : N + 1]
        kc_sb = cst[0:N, N + 1 : N + 3]
        ident = cst[0:N, N + 3 : 2 * N + 3]
        lm_sb = cst[:, 2 * N + 3 : 2 * N + 3 + CB]
        es0_sb = cst[0:N, 2 * N + 3 + CB : 2 * N + 3 + 2 * CB]
        ktf_sb = ctx.enter_context(nc.sbuf_tensor([CB, S * N], FT))
        out_tr = ctx.enter_context(nc.sbuf_tensor([CB, S * N], FT))
        vfull = ctx.enter_context(nc.sbuf_tensor([N, CB], FT))
        vT_sb = ctx.enter_context(nc.sbuf_tensor([CB, N], FT))
        d_sb = ctx.enter_context(nc.sbuf_tensor([CB, 1], FT))
        s_ps = ctx.enter_context(nc.psum_tensor([N, CB], FT))
        tq0 = ctx.enter_context(nc.psum_tensor([CB, 2 * N], FT))
        tq1 = ctx.enter_context(nc.psum_tensor([CB, 2 * N], FT))
        tq2 = ctx.enter_context(nc.psum_tensor([CB, 2 * N], FT))
        tq3 = ctx.enter_context(nc.psum_tensor([CB, 2 * N], FT))
        vt_ps = ctx.enter_context(nc.psum_tensor([CB, N], FT))
        d_ps = ctx.enter_context(nc.psum_tensor([CB, N], FT))
        dma_c = ctx.enter_context(nc.semaphore())
        dma_es = ctx.enter_context(nc.semaphore())
        dma_kt = ctx.enter_context(nc.semaphore())
        acte = ctx.enter_context(nc.semaphore())
        exp_sem = ctx.enter_context(nc.semaphore())
        mset = ctx.enter_context(nc.semaphore())
        dve = ctx.enter_context(nc.semaphore())
        pe = ctx.enter_context(nc.semaphore())
        ln_sem = ctx.enter_context(nc.semaphore())
        tp_sem = ctx.enter_context(nc.semaphore())
        vsub = ctx.enter_context(nc.semaphore())
        tpv = ctx.enter_context(nc.semaphore())
        vtc = ctx.enter_context(nc.semaphore())
        dmm = ctx.enter_context(nc.semaphore())
        dsb = ctx.enter_context(nc.semaphore())
        fin = ctx.enter_context(nc.semaphore())
        outd = ctx.enter_context(nc.semaphore())
        block = ctx.enter_context(nc.Block())
        tq = [tq0, tq1, tq2, tq3]

        @block.sync
        def _(sync):
            sync.dma_start(cst[:, :], cst_d[:, :]).then_inc(dma_c, 16)
            for k in range(1, len(DCUM)):
                sync.dma_start(
                    es_sc[:, DCUM[k - 1] : DCUM[k]], scp_d[:, DCUM[k - 1] : DCUM[k]]
                ).then_inc(dma_es, 16)
            sync.dma_start(ktf_sb[:, :], ktf_d[:, :]).then_inc(dma_kt, 16)
            for k in range(FPIECE):
                sync.wait_ge(fin, k + 1)
                sync.dma_start(
                    out_d[:, k * FC : (k + 1) * FC], out_tr[:, k * FC : (k + 1) * FC]
                ).then_inc(outd, 16)

        def tp_op(tensor, rp):
            # tq bank reuse (h vs h-4) is safe without a wait: the scan
            # matmul before this transpose waited dve>=2h+4, and tqa_op(h-3)
            # precedes mul_{2h+3} in the in-order DVE program.
            h = rp // 2
            t = tensor.transpose(
                tq[h % 4][:, (rp % 2) * N : (rp % 2 + 1) * N],
                lnp[:, rp * CB : (rp + 1) * CB],
                ident[:, :],
            )
            t._wait_ge(ln_sem, h + 1)
            t.then_inc(tp_sem, 1)

        @block.tensor
        def _(tensor):
            tensor.wait_ge(mset, 1)
            for r in range(1, S):
                mm = tensor.matmul(
                    s_ps[:, :], e_sb[:, :], p_all[:, (r - 1) * CB : r * CB]
                )
                mm._wait_ge(dve, r)
                mm.then_inc(pe, 1)
                if r == 4:
                    tensor.wait_ge(dma_c, 16)
                if r >= 4:
                    tp_op(tensor, r - 4)
            for rp in range(S - 4, S):
                tp_op(tensor, rp)
            tv = tensor.transpose(vt_ps[:, :], vfull[:, :], ident[:, :])
            tv._wait_ge(vsub, 1)
            tv.then_inc(tpv, 1)
            dm = tensor.matmul(d_ps[:, :], lm_sb[:, :], vT_sb[:, :])
            dm._wait_ge(vtc, 1)
            dm.then_inc(dmm, 1)

        @block.scalar
        def _(scalar):
            # dummy exp: pull the ACT table load into the runtime-init window
            scalar.activation(scr[:, :], scr[:, :], AF.Exp)
            scalar.wait_ge(dma_c, 16)
            scalar.activation(e_sb[:, :], tr_nat[:, :], AF.Exp).then_inc(acte, 1)
            scalar.activation(
                e0k[:, :], tcol_sb[:, :], AF.Exp, bias=kc_sb[:, 0:1]
            ).then_inc(acte, 1)
            scalar.activation(
                es[:, 0:CB], es0_sb[:, :], AF.Exp, bias=kc_sb[:, 1:2]
            ).then_inc(exp_sem, 1)
            for k in range(1, len(XCUM)):
                gate = next(j for j, c in enumerate(DCUM) if c >= XCUM[k])
                scalar.wait_ge(dma_es, 16 * gate)
                scalar.activation(
                    es[:, XCUM[k - 1] : XCUM[k]],
                    es_sc[:, XCUM[k - 1] : XCUM[k]],
                    AF.Exp,
                    bias=kc_sb[:, 1:2],
                ).then_inc(exp_sem, 1)
            for h in range(S // 2):
                a = scalar.activation(
                    lnp[:, 2 * h * CB : (2 * h + 2) * CB],
                    p_all[:, 2 * h * CB : (2 * h + 2) * CB],
                    AF.Ln,
                )
                a._wait_ge(dve, 2 * h + 2)
                a.then_inc(ln_sem, 1)
            cp1 = scalar.copy(vT_sb[:, :], vt_ps[:, :])
            cp1._wait_ge(tpv, 1)
            cp1.then_inc(vtc, 1)
            cp2 = scalar.copy(d_sb[:, :], d_ps[:, 1:2])
            cp2._wait_ge(dmm, 1)
            cp2.then_inc(dsb, 1)

        def tqa_op(vector, h):
            a = vector.tensor_add(
                out_tr[:, h * 2 * N : (h + 1) * 2 * N],
                tq[h % 4][:, :],
                ktf_sb[:, h * 2 * N : (h + 1) * 2 * N],
            )
            a._wait_ge(tp_sem, 2 * h + 2)

        @block.vector
        def _(vector):
            vector.wait_ge(acte, 1)
            vector.memset(e_sb[:, 0:1], 1.0)
            vector.memset(e_sb[0:1, :], 0.0).then_inc(mset, 1)
            vector.wait_ge(acte, 2)
            vector.memset(e0k[0:1, 0:1], float(np.exp(K)))
            vector.memset(vfull[:, 0:BS], 0.0)
            vector.wait_ge(exp_sem, 1)
            vector.tensor_scalar_mul(
                p_all[:, 0:CB], es[:, 0:CB], e0k[:, :]
            ).then_inc(dve, 1)
            for r in range(1, S):
                if _piece(r) > _piece(r - 1):
                    vector.wait_ge(exp_sem, _piece(r))
                m = vector.tensor_mul(
                    p_all[:, r * CB : (r + 1) * CB],
                    s_ps[:, :],
                    es[:, r * CB : (r + 1) * CB],
                )
                m._wait_ge(pe, r)
                m.then_inc(dve, 1)
                if r == 5:
                    vector.wait_ge(dma_kt, 16)
                if r >= 5 and (r - 5) % 2 == 0:
                    tqa_op(vector, (r - 5) // 2)
            for h in range((S - 5) // 2 + 1, S // 2):
                tqa_op(vector, h)
            sub = vector.tensor_sub(
                vfull[:, BS:CB],
                lnp[:, (S - 1) * CB : (S - 1) * CB + (C - 1) * BS],
                lnp[:, (W - 1) * CB + BS : (W - 1) * CB + CB],
            )
            sub._wait_ge(ln_sem, S // 2)
            sub.then_inc(vsub, 1)
            for k in range(FPIECE):
                f = vector.tensor_scalar_add(
                    out_tr[:, k * FC : (k + 1) * FC],
                    out_tr[:, k * FC : (k + 1) * FC],
                    d_sb[:, :],
                )
                if k == 0:
                    f._wait_ge(dsb, 1)
                f.then_inc(fin, 1)

    return nc


LAST_RESULT = None


def kernel(scores: np.ndarray, transitions: np.ndarray) -> np.ndarray:
    global LAST_RESULT
    from concourse.bass_utils import run_bass_kernel_spmd

    scores = np.ascontiguousarray(scores, dtype=np.float32)
    transitions = np.ascontiguousarray(transitions, dtype=np.float32)

    # host-side constants and layout permutes (no math on the data path)
    idx_t = np.arange(C)[None, :] * L + np.arange(S)[:, None]      # (S, C)
    consts = np.zeros((CB, 2 * N + 3 + 2 * CB), np.float32)
    consts[0:N, 0:N] = transitions
    consts[0:N, N] = transitions[0, :]
    consts[0:N, N + 1] = K
    consts[0:N, N + 2] = -K
    consts[0:N, N + 3 : 2 * N + 3] = np.eye(N, dtype=np.float32)
    ces0 = 2 * N + 3 + CB
    cidx = np.repeat(np.arange(C), BS)
    bidx = np.tile(np.arange(BS), C)
    M = (
        (bidx[:, None] == bidx[None, :])
        & (cidx[None, :] >= 1)
        & (cidx[None, :] <= cidx[:, None])
    ).astype(np.float32)
    consts[:, 2 * N + 3 : 2 * N + 3 + CB] = M.T
    tvals = (np.arange(C)[:, None] * L + np.arange(S)[None, :]).astype(np.float32)
    ktf = np.repeat(K * tvals[:, None, :], BS, axis=1).reshape(CB, S)
    ktfull = np.repeat(ktf[:, :, None], N, axis=2).reshape(CB, S * N)
    ktfull[:, 0::N] -= 10000.0
    ktfull = np.ascontiguousarray(ktfull)

    nc = _build_program()
    in_maps = []
    for g in range(NCORES):
        blk = scores[g * BS : (g + 1) * BS]                 # (BS, T, N)
        scp = np.ascontiguousarray(
            blk[:, idx_t, :].transpose(3, 1, 2, 0).reshape(N, NCOL)
        )
        cst_g = consts.copy()
        cst_g[0:N, ces0 : ces0 + CB] = scp[:, 0:CB]
        in_maps.append(
            {"scp": scp, "consts": cst_g, "ktfull": ktfull}
        )
    res = run_bass_kernel_spmd(nc, in_maps, list(range(NCORES)))
    LAST_RESULT = res
    out = np.empty((B, T, N), dtype=np.float32)
    for g in range(NCORES):
        arr = res.results[g]["out"].reshape(C, BS, S, N)
        og = out[g * BS : (g + 1) * BS]
        og[:, 0:S] = arr[0]
        for c in range(1, C):
            og[:, c * L + W : c * L + S] = arr[c, :, W:S]
    return out



# revision 9
# speedup vs baseline: 1.0885x; 1.0885x over previous
"""CRF forward-algorithm kernel for Trainium2 (8 NeuronCores, Bass).

Strategy: data-parallel over batch (32 -> 4 per core) PLUS chunked-scan
parallelism over time. The recursion

    alpha_t[b,j] = scores[b,t,j] + lse_i(trans[i,j] + alpha_{t-1}[b,i])

is run in linear space with a global per-step normalizer K:

    p_t[j,(c,b)] = exp(scores - K) * sum_i E[i,j] p_{t-1}[i,(c,b)]

E = exp(trans) has entries in [e^-0.1, e^0.1], so the linear map contracts
the Hilbert projective metric by ~0.1 per step; a W=2 warmup from a
surrogate init (alpha ~ emission scores) leaves only ~1e-2 absolute error,
far inside the 2e-2-relative budget (output scale ~1e4). T=512 is split
into C=30 chunks of L=17 steps, warm-started W=2 steps early; sequential
scan length S = L+W = 19. Only a per-chunk scalar log-offset delta remains,
recovered by a DVE tensor_tensor_scan prefix over chunk-boundary
mismatches and added per-partition to the transposed output.

Per step: one PE matmul (E stationary bf16, 120 moving columns) + one DVE
multiply. ln/transposes/correction pipeline behind the scan on ACT/PE/DVE;
the output leaves in a [(chunk,batch) partition, (step,tag)] layout where
K*t - 10000*[j==0] is a host-precomputed fp16 constant fused into the
PSUM->SBUF copy. Host does only layout permutes (gather/scatter), no math.
"""

import numpy as np

N = 64
T = 512
B = 32
NCORES = 8
BS = B // NCORES   # 4 batch elements per core
C = 30             # time chunks
W = 2              # warmup steps per chunk
L = (T - W) // C   # 17 real steps per chunk (chunk 0: L+W)
S = L + W          # 19 sequential scan steps
CB = C * BS        # 120 columns per scan step
NCOL = S * CB      # 2280 state columns
K = 4.66
NPAIR = S // 2     # 9 transposed pairs; step S-1 rides alone
SN = S * N         # 1216 output cols per partition
# scp arrives in 3 DMA pieces; es exp in 5 ops staged behind them
DCUM = [CB, 1080, NCOL]              # cumulative scp cols per DMA completion
XCUM = [CB, 600, 1080, 1680, NCOL]   # cumulative cols per exp op
FPIECE = 2
FC = SN // FPIECE  # 608 cols per final delta-add/DMA-out piece


def _sa_gate(r):
    # ACT-counter value needed before scan column-slice r is read.
    # sA: 1=e0k 2=es piece1 3=e_sb 4..7=es pieces 2..5, 8+h=ln pair h done
    need = (r + 1) * CB
    for i, c in enumerate(XCUM):
        if c >= need:
            return 2 if i == 0 else 3 + i
    return 7


def _build_program():
    import concourse.bass as bass
    import concourse.mybir as mybir

    FT = mybir.dt.float32
    HF = mybir.dt.float16
    BF = mybir.dt.bfloat16
    AF = mybir.ActivationFunctionType
    ALU = mybir.AluOpType

    nc = bass.Bass()
    scp_d = nc.declare_dram_parameter("scp", [N, NCOL], FT, isOutput=False)
    csth_d = nc.declare_dram_parameter("csth", [N, N + 3], FT, isOutput=False)
    cstc_d = nc.declare_dram_parameter("cstc", [N, N], FT, isOutput=False)
    ktf_d = nc.declare_dram_parameter("ktfull", [CB, SN], HF, isOutput=False)
    out_d = nc.declare_dram_parameter("out", [CB, SN], FT, isOutput=True)

    from contextlib import ExitStack

    with ExitStack() as ctx:
        es_sc = ctx.enter_context(nc.sbuf_tensor([N, NCOL], FT))
        es = ctx.enter_context(nc.sbuf_tensor([N, NCOL], FT))
        p_all = ctx.enter_context(nc.sbuf_tensor([N, NCOL], BF))
        lnp = ctx.enter_context(nc.sbuf_tensor([N, NCOL], FT))
        e_sb = ctx.enter_context(nc.sbuf_tensor([N, N], BF))
        csth = ctx.enter_context(nc.sbuf_tensor([N, N + 3], FT))
        ident = ctx.enter_context(nc.sbuf_tensor([N, N], FT))
        scr = ctx.enter_context(nc.sbuf_tensor([N, 1], FT))
        e0k = ctx.enter_context(nc.sbuf_tensor([N, 1], FT))
        ktf_sb = ctx.enter_context(nc.sbuf_tensor([CB, SN], HF))
        out_tr = ctx.enter_context(nc.sbuf_tensor([CB, SN], FT))
        drow = ctx.enter_context(nc.sbuf_tensor([1, CB], FT))
        d_sb = ctx.enter_context(nc.sbuf_tensor([CB, 1], FT))
        tr_nat = csth[0:N, 0:N]
        tcol_sb = csth[0:N, N : N + 1]
        kc_sb = csth[0:N, N + 1 : N + 3]
        s_ps = ctx.enter_context(nc.psum_tensor([N, CB], FT))
        tq0 = ctx.enter_context(nc.psum_tensor([CB, 2 * N], FT))
        tq1 = ctx.enter_context(nc.psum_tensor([CB, 2 * N], FT))
        tq2 = ctx.enter_context(nc.psum_tensor([CB, 2 * N], FT))
        tq3 = ctx.enter_context(nc.psum_tensor([CB, 2 * N], FT))
        tq9 = ctx.enter_context(nc.psum_tensor([CB, N], FT))
        d_ps = ctx.enter_context(nc.psum_tensor([CB, 1], FT))
        dma_h = ctx.enter_context(nc.semaphore())
        dma_s = ctx.enter_context(nc.semaphore())
        dma_c = ctx.enter_context(nc.semaphore())
        dma_kt = ctx.enter_context(nc.semaphore())
        sA = ctx.enter_context(nc.semaphore())
        mset = ctx.enter_context(nc.semaphore())
        dve = ctx.enter_context(nc.semaphore())
        pe = ctx.enter_context(nc.semaphore())
        tp_sem = ctx.enter_context(nc.semaphore())
        st = ctx.enter_context(nc.semaphore())
        fin = ctx.enter_context(nc.semaphore())
        outd = ctx.enter_context(nc.semaphore())
        block = ctx.enter_context(nc.Block())
        tq = [tq0, tq1, tq2, tq3]

        @block.sync
        def _(sync):
            sync.dma_start(csth[:, :], csth_d[:, :]).then_inc(dma_h, 16)
            sync.dma_start(
                es_sc[:, 0:DCUM[0]], scp_d[:, 0:DCUM[0]]
            ).then_inc(dma_s, 16)
            sync.dma_start(ident[:, :], cstc_d[:, :]).then_inc(dma_c, 16)
            sync.dma_start(
                es_sc[:, DCUM[0] : DCUM[1]], scp_d[:, DCUM[0] : DCUM[1]]
            ).then_inc(dma_s, 16)
            sync.dma_start(ktf_sb[:, :], ktf_d[:, :]).then_inc(dma_kt, 16)
            sync.dma_start(
                es_sc[:, DCUM[1] : DCUM[2]], scp_d[:, DCUM[1] : DCUM[2]]
            ).then_inc(dma_s, 16)
            for k in range(FPIECE):
                sync.wait_ge(fin, k + 1)
                sync.dma_start(
                    out_d[:, k * FC : (k + 1) * FC], out_tr[:, k * FC : (k + 1) * FC]
                ).then_inc(outd, 16)

        def tp_op(tensor, rp):
            # tq bank reuse (h vs h-4) is safe without a wait: the scan
            # matmul before this transpose waited dve>=r, and tqa_op(h-4)
            # precedes mul_r in the in-order DVE program.
            h = rp // 2
            dst = tq9[:, :] if rp == S - 1 else tq[h % 4][
                :, (rp % 2) * N : (rp % 2 + 1) * N
            ]
            t = tensor.transpose(dst, lnp[:, rp * CB : (rp + 1) * CB], ident[:, :])
            t._wait_ge(sA, 8 + h)
            t.then_inc(tp_sem, 1)

        @block.tensor
        def _(tensor):
            tensor.wait_ge(mset, 1)
            for r in range(1, S):
                mm = tensor.matmul(
                    s_ps[:, :], e_sb[:, :], p_all[:, (r - 1) * CB : r * CB]
                )
                mm._wait_ge(dve, r)
                mm.then_inc(pe, 1)
                if r == 4:
                    tensor.wait_ge(dma_c, 16)
                if r >= 4:
                    tp_op(tensor, r - 4)
            for rp in range(S - 4, S):
                tp_op(tensor, rp)
            tv = tensor.transpose(d_ps[:, :], drow[0:1, :], ident[0:1, 0:1])
            tv._wait_ge(st, 4)
            tv.then_inc(st, 1)

        @block.scalar
        def _(scalar):
            # dummy exp: pull the ACT table load into the runtime-init window
            scalar.activation(scr[:, :], scr[:, :], AF.Exp)
            scalar.wait_ge(dma_h, 16)
            scalar.activation(
                e0k[:, :], tcol_sb[:, :], AF.Exp, bias=kc_sb[:, 0:1]
            ).then_inc(sA, 1)
            scalar.wait_ge(dma_s, 16)
            scalar.activation(
                es[:, 0:CB], es_sc[:, 0:CB], AF.Exp, bias=kc_sb[:, 1:2]
            ).then_inc(sA, 1)
            scalar.activation(e_sb[:, :], tr_nat[:, :], AF.Exp).then_inc(sA, 1)
            for k in range(1, len(XCUM)):
                gate = next(j for j, c in enumerate(DCUM) if c >= XCUM[k])
                scalar.wait_ge(dma_s, 16 * (gate + 1))
                scalar.activation(
                    es[:, XCUM[k - 1] : XCUM[k]],
                    es_sc[:, XCUM[k - 1] : XCUM[k]],
                    AF.Exp,
                    bias=kc_sb[:, 1:2],
                ).then_inc(sA, 1)
            for h in range(NPAIR):
                a = scalar.activation(
                    lnp[:, 2 * h * CB : (2 * h + 2) * CB],
                    p_all[:, 2 * h * CB : (2 * h + 2) * CB],
                    AF.Ln,
                )
                a._wait_ge(dve, 2 * h + 2)
                a.then_inc(sA, 1)
            a = scalar.activation(
                lnp[:, (S - 1) * CB : S * CB],
                p_all[:, (S - 1) * CB : S * CB],
                AF.Ln,
            )
            a._wait_ge(dve, S)
            a.then_inc(sA, 1)
            cp = scalar.copy(d_sb[:, :], d_ps[:, :])
            cp._wait_ge(st, 5)
            cp.then_inc(st, 1)

        def tqa_op(vector, h):
            if h == NPAIR:
                a = vector.tensor_add(
                    out_tr[:, h * 2 * N : h * 2 * N + N],
                    tq9[:, :],
                    ktf_sb[:, h * 2 * N : h * 2 * N + N],
                )
            else:
                a = vector.tensor_add(
                    out_tr[:, h * 2 * N : (h + 1) * 2 * N],
                    tq[h % 4][:, :],
                    ktf_sb[:, h * 2 * N : (h + 1) * 2 * N],
                )
            a._wait_ge(tp_sem, 2 * h + 2 if h < NPAIR else S)

        @block.vector
        def _(vector):
            vector.memset(drow[0:1, 0:BS], 0.0)
            vector.wait_ge(sA, 1)
            vector.memset(e0k[0:1, 0:1], float(np.exp(K)))
            vector.wait_ge(sA, 2)
            vector.tensor_scalar_mul(
                p_all[:, 0:CB], es[:, 0:CB], e0k[:, :]
            ).then_inc(dve, 1)
            vector.wait_ge(sA, 3)
            vector.memset(e_sb[:, 0:1], 1.0)
            vector.memset(e_sb[0:1, :], 0.0).then_inc(mset, 1)
            for r in range(1, S):
                if _sa_gate(r) > _sa_gate(r - 1):
                    vector.wait_ge(sA, _sa_gate(r))
                m = vector.tensor_mul(
                    p_all[:, r * CB : (r + 1) * CB],
                    s_ps[:, :],
                    es[:, r * CB : (r + 1) * CB],
                )
                m._wait_ge(pe, r)
                m.then_inc(dve, 1)
                if r == 5:
                    vector.wait_ge(dma_kt, 16)
                if r >= 5 and (r - 5) % 2 == 0:
                    tqa_op(vector, (r - 5) // 2)
            for h in range((S - 6) // 2 + 1, NPAIR + 1):
                tqa_op(vector, h)
            # START column: alpha[...,0] carries a -10000 from transitions[:,0]
            vector.tensor_scalar_add(
                out_tr[:, 0:SN:N], out_tr[:, 0:SN:N], -10000.0
            )
            # stitch: delta[c,b] = prefix_c(lnp[S-1,0,(c-1),b] - lnp[W-1,0,c,b])
            for b in range(BS):
                sc0 = (S - 1) * CB + b
                sc1 = (W - 1) * CB + BS + b
                t = vector.tensor_tensor_scan(
                    drow[0:1, BS + b : CB : BS],
                    lnp[0:1, sc0 : sc0 + (C - 1) * BS : BS],
                    lnp[0:1, sc1 : sc1 + (C - 1) * BS : BS],
                    0.0,
                    ALU.add,
                    ALU.subtract,
                )
                if b == 0:
                    t._wait_ge(sA, 8 + NPAIR)
                t.then_inc(st, 1)
            for k in range(FPIECE):
                f = vector.tensor_scalar_add(
                    out_tr[:, k * FC : (k + 1) * FC],
                    out_tr[:, k * FC : (k + 1) * FC],
                    d_sb[:, :],
                )
                if k == 0:
                    f._wait_ge(st, 6)
                f.then_inc(fin, 1)

    return nc


LAST_RESULT = None


def kernel(scores: np.ndarray, transitions: np.ndarray) -> np.ndarray:
    global LAST_RESULT
    from concourse.bass_utils import run_bass_kernel_spmd

    scores = np.ascontiguousarray(scores, dtype=np.float32)
    transitions = np.ascontiguousarray(transitions, dtype=np.float32)

    # host-side constants and layout permutes (no math on the data path)
    idx_t = np.arange(C)[None, :] * L + np.arange(S)[:, None]      # (S, C)
    csth = np.zeros((N, N + 3), np.float32)
    csth[:, 0:N] = transitions
    csth[:, N] = transitions[0, :]
    csth[:, N + 1] = K
    csth[:, N + 2] = -K
    cstc = np.eye(N, dtype=np.float32)
    tvals = (np.arange(C)[:, None] * L + np.arange(S)[None, :]).astype(np.float32)
    ktf = np.repeat(K * tvals[:, None, :], BS, axis=1).reshape(CB, S)
    ktfull = np.repeat(ktf[:, :, None], N, axis=2).reshape(CB, SN)
    ktfull = np.ascontiguousarray(ktfull.astype(np.float16))

    nc = _build_program()
    in_maps = []
    for g in range(NCORES):
        blk = scores[g * BS : (g + 1) * BS]                 # (BS, T, N)
        scp = np.ascontiguousarray(
            blk[:, idx_t, :].transpose(3, 1, 2, 0).reshape(N, NCOL)
        )
        in_maps.append(
            {"scp": scp, "csth": csth, "cstc": cstc, "ktfull": ktfull}
        )
    res = run_bass_kernel_spmd(nc, in_maps, list(range(NCORES)))
    LAST_RESULT = res
    out = np.empty((B, T, N), dtype=np.float32)
    for g in range(NCORES):
        arr = res.results[g]["out"].reshape(C, BS, S, N)
        og = out[g * BS : (g + 1) * BS]
        og[:, 0:S] = arr[0]
        for c in range(1, C):
            og[:, c * L + W : c * L + S] = arr[c, :, W:S]
    return out


# revision 13
# speedup vs baseline: 1.0951x; 1.0060x over previous
"""CRF forward-algorithm kernel for Trainium2 (8 NeuronCores, Bass).

Strategy: data-parallel over batch (32 -> 4 per core) PLUS chunked-scan
parallelism over time. The recursion

    alpha_t[b,j] = scores[b,t,j] + lse_i(trans[i,j] + alpha_{t-1}[b,i])

is run in linear space with a global per-step normalizer K:

    p_t[j,(c,b)] = exp(scores - K) * sum_i E[i,j] p_{t-1}[i,(c,b)]

E = exp(trans) has entries in [e^-0.1, e^0.1], so the linear map contracts
the Hilbert projective metric by ~0.1 per step; a W=2 warmup from a
surrogate init (alpha ~ emission scores) leaves only ~1e-2 absolute error,
far inside the 2e-2-relative budget (output scale ~1e4). T=512 is split
into C=30 chunks of L=17 steps, warm-started W=2 steps early; sequential
scan length S = L+W = 19. Only a per-chunk scalar log-offset delta remains,
recovered by a DVE tensor_tensor_scan prefix over chunk-boundary
mismatches and added per-partition to the transposed output.

Engine layout: PE runs the scan matmuls with the output transposes
interleaved in its idle slots; ACT does exp/ln; DVE does the scan
multiplies and the delta stitch; GPSIMD (otherwise idle) does the
PSUM->SBUF moves fused with the K*t fp16 constant add, keeping a late
ktf DMA off the scan's critical path. The E fixup (column 0 -> 1, row
0 -> 0) is baked into the host constants as 0.0 / -10000.0 entries so
exp() produces it directly - no on-device memsets. All input DMAs share
one in-order completion semaphore. Host does only layout permutes.
"""

import numpy as np

N = 64
T = 512
B = 32
NCORES = 8
BS = B // NCORES   # 4 batch elements per core
C = 30             # time chunks
W = 2              # warmup steps per chunk
L = (T - W) // C   # 17 real steps per chunk (chunk 0: L+W)
S = L + W          # 19 sequential scan steps
CB = C * BS        # 120 columns per scan step
NCOL = S * CB      # 2280 state columns
K = 4.66
NPAIR = S // 2     # 9 transposed pairs; step S-1 rides alone
SN = S * N         # 1216 output cols per partition
XCUM = [CB, 720, 1560, NCOL]   # cumulative cols per es exp op
FPIECE = 2
FC = SN // FPIECE  # 608 cols per final delta-add/DMA-out piece
# merged DMA sem thresholds (issue order = completion order on the queue)
D_CSTH, D_SCP0, D_SCP1, D_IDENT, D_KTF, D_SCP2, D_SCP3 = (
    16, 32, 48, 64, 80, 96, 112)


def _sa_gate(r):
    # ACT-counter value needed before scan column-slice r is read.
    # sA: 1=e0k 2=es piece1 3=e_sb 4..6=es pieces 2..4, 7+h=ln pair h done
    need = (r + 1) * CB
    for i, c in enumerate(XCUM):
        if c >= need:
            return 2 if i == 0 else 3 + i
    return 6


def _build_program():
    import concourse.bass as bass
    import concourse.mybir as mybir

    FT = mybir.dt.float32
    HF = mybir.dt.float16
    BF = mybir.dt.bfloat16
    AF = mybir.ActivationFunctionType
    ALU = mybir.AluOpType

    nc = bass.Bass()
    scp_d = nc.declare_dram_parameter("scp", [N, NCOL], FT, isOutput=False)
    csth_d = nc.declare_dram_parameter("csth", [N, N + 3], FT, isOutput=False)
    cstc_d = nc.declare_dram_parameter("cstc", [N, N], FT, isOutput=False)
    ktf_d = nc.declare_dram_parameter("ktfull", [CB, SN], HF, isOutput=False)
    out_d = nc.declare_dram_parameter("out", [CB, SN], FT, isOutput=True)

    from contextlib import ExitStack

    with ExitStack() as ctx:
        es_sc = ctx.enter_context(nc.sbuf_tensor([N, NCOL], FT))
        es = ctx.enter_context(nc.sbuf_tensor([N, NCOL], FT))
        p_all = ctx.enter_context(nc.sbuf_tensor([N, NCOL], BF))
        lnp = ctx.enter_context(nc.sbuf_tensor([N, NCOL], FT))
        e_sb = ctx.enter_context(nc.sbuf_tensor([N, N], BF))
        csth = ctx.enter_context(nc.sbuf_tensor([N, N + 3], FT))
        ident = ctx.enter_context(nc.sbuf_tensor([N, N], FT))
        scr = ctx.enter_context(nc.sbuf_tensor([N, 1], FT))
        e0k = ctx.enter_context(nc.sbuf_tensor([N, 1], FT))
        ktf_sb = ctx.enter_context(nc.sbuf_tensor([CB, SN], HF))
        out_tr = ctx.enter_context(nc.sbuf_tensor([CB, SN], FT))
        drow = ctx.enter_context(nc.sbuf_tensor([1, CB], FT))
        d_sb = ctx.enter_context(nc.sbuf_tensor([CB, 1], FT))
        tr_nat = csth[0:N, 0:N]
        tcol_sb = csth[0:N, N : N + 1]
        kc_sb = csth[0:N, N + 1 : N + 3]
        s_ps = ctx.enter_context(nc.psum_tensor([N, CB], FT))
        tq0 = ctx.enter_context(nc.psum_tensor([CB, 2 * N], FT))
        tq1 = ctx.enter_context(nc.psum_tensor([CB, 2 * N], FT))
        tq2 = ctx.enter_context(nc.psum_tensor([CB, 2 * N], FT))
        tq3 = ctx.enter_context(nc.psum_tensor([CB, 2 * N], FT))
        tq9 = ctx.enter_context(nc.psum_tensor([CB, N], FT))
        d_ps = ctx.enter_context(nc.psum_tensor([CB, 1], FT))
        dma = ctx.enter_context(nc.semaphore())
        sA = ctx.enter_context(nc.semaphore())
        dve = ctx.enter_context(nc.semaphore())
        pe = ctx.enter_context(nc.semaphore())
        tp_sem = ctx.enter_context(nc.semaphore())
        gpa = ctx.enter_context(nc.semaphore())
        st = ctx.enter_context(nc.semaphore())
        fin = ctx.enter_context(nc.semaphore())
        outd = ctx.enter_context(nc.semaphore())
        block = ctx.enter_context(nc.Block())
        tq = [tq0, tq1, tq2, tq3]

        @block.sync
        def _(sync):
            sync.dma_start(csth[:, :], csth_d[:, :]).then_inc(dma, 16)
            sync.dma_start(
                es_sc[:, 0:XCUM[0]], scp_d[:, 0:XCUM[0]]
            ).then_inc(dma, 16)
            sync.dma_start(
                es_sc[:, XCUM[0] : XCUM[1]], scp_d[:, XCUM[0] : XCUM[1]]
            ).then_inc(dma, 16)
            sync.dma_start(ident[:, :], cstc_d[:, :]).then_inc(dma, 16)
            sync.dma_start(ktf_sb[:, :], ktf_d[:, :]).then_inc(dma, 16)
            sync.dma_start(
                es_sc[:, XCUM[1] : XCUM[2]], scp_d[:, XCUM[1] : XCUM[2]]
            ).then_inc(dma, 16)
            sync.dma_start(
                es_sc[:, XCUM[2] : XCUM[3]], scp_d[:, XCUM[2] : XCUM[3]]
            ).then_inc(dma, 16)
            for k in range(FPIECE):
                sync.wait_ge(fin, k + 1)
                sync.dma_start(
                    out_d[:, k * FC : (k + 1) * FC], out_tr[:, k * FC : (k + 1) * FC]
                ).then_inc(outd, 16)

        def tp_op(tensor, rp):
            # tq bank h%4 is freed by GPSIMD's tqa_op(h-4): gate on gpa.
            h = rp // 2
            dst = tq9[:, :] if rp == S - 1 else tq[h % 4][
                :, (rp % 2) * N : (rp % 2 + 1) * N
            ]
            if rp % 2 == 0 and 4 <= h < NPAIR:
                tensor.wait_ge(gpa, h - 3)
            t = tensor.transpose(dst, lnp[:, rp * CB : (rp + 1) * CB], ident[:, :])
            t._wait_ge(sA, 7 + h)
            t.then_inc(tp_sem, 1)

        @block.tensor
        def _(tensor):
            tensor.wait_ge(sA, 3)
            for r in range(1, S):
                mm = tensor.matmul(
                    s_ps[:, :], e_sb[:, :], p_all[:, (r - 1) * CB : r * CB]
                )
                mm._wait_ge(dve, r)
                mm.then_inc(pe, 1)
                if r == 4:
                    tensor.wait_ge(dma, D_IDENT)
                if r >= 4:
                    tp_op(tensor, r - 4)
            for rp in range(S - 4, S):
                tp_op(tensor, rp)
            tv = tensor.transpose(d_ps[:, :], drow[0:1, :], ident[0:1, 0:1])
            tv._wait_ge(st, 4)
            tv.then_inc(st, 1)

        @block.scalar
        def _(scalar):
            # dummy exp: pull the ACT table load into the runtime-init window
            scalar.activation(scr[:, :], scr[:, :], AF.Exp)
            scalar.wait_ge(dma, D_CSTH)
            scalar.activation(
                e0k[:, :], tcol_sb[:, :], AF.Exp, bias=kc_sb[:, 0:1]
            ).then_inc(sA, 1)
            scalar.wait_ge(dma, D_SCP0)
            scalar.activation(
                es[:, 0:CB], es_sc[:, 0:CB], AF.Exp, bias=kc_sb[:, 1:2]
            ).then_inc(sA, 1)
            scalar.activation(e_sb[:, :], tr_nat[:, :], AF.Exp).then_inc(sA, 1)
            for k, gate in ((1, D_SCP1), (2, D_SCP2), (3, D_SCP3)):
                scalar.wait_ge(dma, gate)
                scalar.activation(
                    es[:, XCUM[k - 1] : XCUM[k]],
                    es_sc[:, XCUM[k - 1] : XCUM[k]],
                    AF.Exp,
                    bias=kc_sb[:, 1:2],
                ).then_inc(sA, 1)
            for h in range(NPAIR):
                a = scalar.activation(
                    lnp[:, 2 * h * CB : (2 * h + 2) * CB],
                    p_all[:, 2 * h * CB : (2 * h + 2) * CB],
                    AF.Ln,
                )
                a._wait_ge(dve, 2 * h + 2)
                a.then_inc(sA, 1)
            a = scalar.activation(
                lnp[:, (S - 1) * CB : S * CB],
                p_all[:, (S - 1) * CB : S * CB],
                AF.Ln,
            )
            a._wait_ge(dve, S)
            a.then_inc(sA, 1)
            cp = scalar.copy(d_sb[:, :], d_ps[:, :])
            cp._wait_ge(st, 5)
            cp.then_inc(st, 1)

        def tqa_op(vector, h):
            # PSUM->SBUF move fused with the K*t fp16 constant add
            if h == NPAIR:
                a = vector.tensor_add(
                    out_tr[:, h * 2 * N : h * 2 * N + N],
                    tq9[:, :],
                    ktf_sb[:, h * 2 * N : h * 2 * N + N],
                )
                a._wait_ge(tp_sem, S)
            else:
                a = vector.tensor_add(
                    out_tr[:, h * 2 * N : (h + 1) * 2 * N],
                    tq[h % 4][:, :],
                    ktf_sb[:, h * 2 * N : (h + 1) * 2 * N],
                )
                a._wait_ge(tp_sem, 2 * h + 2)
            a.then_inc(gpa, 1)

        @block.vector
        def _(vector):
            vector.memset(drow[0:1, 0:BS], 0.0)
            vector.wait_ge(sA, 2)
            vector.tensor_scalar_mul(
                p_all[:, 0:CB], es[:, 0:CB], e0k[:, :]
            ).then_inc(dve, 1)
            for r in range(1, S):
                if _sa_gate(r) > _sa_gate(r - 1):
                    vector.wait_ge(sA, _sa_gate(r))
                m = vector.tensor_mul(
                    p_all[:, r * CB : (r + 1) * CB],
                    s_ps[:, :],
                    es[:, r * CB : (r + 1) * CB],
                )
                m._wait_ge(pe, r)
                m.then_inc(dve, 1)
                if r == 9:
                    vector.wait_ge(dma, D_KTF)
                if r >= 9 and (r - 9) % 2 == 0:
                    tqa_op(vector, (r - 9) // 2)
            for h in range((S - 10) // 2 + 1, NPAIR + 1):
                tqa_op(vector, h)
            # START column: alpha[...,0] carries a -10000 from transitions[:,0]
            vector.tensor_scalar_add(
                out_tr[:, 0:SN:N], out_tr[:, 0:SN:N], -10000.0
            )
            # stitch: delta[c,b] = prefix_c(lnp[S-1,0,(c-1),b] - lnp[W-1,0,c,b])
            for b in range(BS):
                sc0 = (S - 1) * CB + b
                sc1 = (W - 1) * CB + BS + b
                t = vector.tensor_tensor_scan(
                    drow[0:1, BS + b : CB : BS],
                    lnp[0:1, sc0 : sc0 + (C - 1) * BS : BS],
                    lnp[0:1, sc1 : sc1 + (C - 1) * BS : BS],
                    0.0,
                    ALU.add,
                    ALU.subtract,
                )
                if b == 0:
                    t._wait_ge(sA, 7 + NPAIR)
                t.then_inc(st, 1)
            for k in range(FPIECE):
                f = vector.tensor_scalar_add(
                    out_tr[:, k * FC : (k + 1) * FC],
                    out_tr[:, k * FC : (k + 1) * FC],
                    d_sb[:, :],
                )
                if k == 0:
                    f._wait_ge(st, 6)
                f.then_inc(fin, 1)

    return nc


LAST_RESULT = None


def kernel(scores: np.ndarray, transitions: np.ndarray) -> np.ndarray:
    global LAST_RESULT
    from concourse.bass_utils import run_bass_kernel_spmd

    scores = np.ascontiguousarray(scores, dtype=np.float32)
    transitions = np.ascontiguousarray(transitions, dtype=np.float32)

    # host-side constants and layout permutes (no math on the data path)
    idx_t = np.arange(C)[None, :] * L + np.arange(S)[:, None]      # (S, C)
    csth = np.zeros((N, N + 3), np.float32)
    csth[:, 0:N] = transitions
    # E fixup baked into the table: exp(0)=1 on column 0, exp(-1e4)=0 on row 0
    csth[:, 0] = 0.0
    csth[0, 0:N] = -10000.0
    csth[:, N] = transitions[0, :]
    csth[0, N] = 0.0               # e0k[0] = exp(K)
    csth[:, N + 1] = K
    csth[:, N + 2] = -K
    cstc = np.eye(N, dtype=np.float32)
    tvals = (np.arange(C)[:, None] * L + np.arange(S)[None, :]).astype(np.float32)
    ktf = np.repeat(K * tvals[:, None, :], BS, axis=1).reshape(CB, S)
    ktfull = np.repeat(ktf[:, :, None], N, axis=2).reshape(CB, SN)
    ktfull = np.ascontiguousarray(ktfull.astype(np.float16))

    nc = _build_program()
    in_maps = []
    for g in range(NCORES):
        blk = scores[g * BS : (g + 1) * BS]                 # (BS, T, N)
        scp = np.ascontiguousarray(
            blk[:, idx_t, :].transpose(3, 1, 2, 0).reshape(N, NCOL)
        )
        in_maps.append(
            {"scp": scp, "csth": csth, "cstc": cstc, "ktfull": ktfull}
        )
    res = run_bass_kernel_spmd(nc, in_maps, list(range(NCORES)))
    LAST_RESULT = res
    out = np.empty((B, T, N), dtype=np.float32)
    for g in range(NCORES):
        arr = res.results[g]["out"].reshape(C, BS, S, N)
        og = out[g * BS : (g + 1) * BS]
        og[:, 0:S] = arr[0]
        for c in range(1, C):
            og[:, c * L + W : c * L + S] = arr[c, :, W:S]
    return out


# revision 14
# speedup vs baseline: 1.0986x; 1.0032x over previous
"""CRF forward-algorithm kernel for Trainium2 (8 NeuronCores, Bass).

Strategy: data-parallel over batch (32 -> 4 per core) PLUS chunked-scan
parallelism over time. The recursion

    alpha_t[b,j] = scores[b,t,j] + lse_i(trans[i,j] + alpha_{t-1}[b,i])

is run in linear space with a global per-step normalizer K:

    p_t[j,(c,b)] = exp(scores - K) * sum_i E[i,j] p_{t-1}[i,(c,b)]

E = exp(trans) has entries in [e^-0.1, e^0.1], so the linear map contracts
the Hilbert projective metric by ~0.1 per step; a W=2 warmup from a
surrogate init (alpha ~ emission scores) leaves only ~1e-2 absolute error,
far inside the 2e-2-relative budget (output scale ~1e4). T=512 is split
into C=30 chunks of L=17 steps, warm-started W=2 steps early; sequential
scan length S = L+W = 19. Only a per-chunk scalar log-offset delta remains,
recovered by a DVE tensor_tensor_scan prefix over chunk-boundary
mismatches and added per-partition to the transposed output.

Engine layout: PE runs the scan matmuls with the output transposes
interleaved in its idle slots; ACT does exp/ln; DVE does the scan
multiplies and the delta stitch; GPSIMD (otherwise idle) does the
PSUM->SBUF moves fused with the K*t fp16 constant add, keeping a late
ktf DMA off the scan's critical path. The E fixup (column 0 -> 1, row
0 -> 0) is baked into the host constants as 0.0 / -10000.0 entries so
exp() produces it directly - no on-device memsets. All input DMAs share
one in-order completion semaphore. Host does only layout permutes.
"""

import numpy as np

N = 64
T = 512
B = 32
NCORES = 8
BS = B // NCORES   # 4 batch elements per core
C = 30             # time chunks
W = 2              # warmup steps per chunk
L = (T - W) // C   # 17 real steps per chunk (chunk 0: L+W)
S = L + W          # 19 sequential scan steps
CB = C * BS        # 120 columns per scan step
NCOL = S * CB      # 2280 state columns
K = 4.66
NPAIR = S // 2     # 9 transposed pairs; step S-1 rides alone
SN = S * N         # 1216 output cols per partition
XCUM = [CB, 720, 1560, NCOL]   # cumulative cols per es exp op
FPIECE = 2
FC = SN // FPIECE  # 608 cols per final delta-add/DMA-out piece



def _sa_gate(r):
    # ACT-counter value needed before scan column-slice r is read.
    # sA: 1=e0k 2=es piece1 3=e_sb 4..6=es pieces 2..4, 7+h=ln pair h done
    need = (r + 1) * CB
    for i, c in enumerate(XCUM):
        if c >= need:
            return 2 if i == 0 else 3 + i
    return 6


def _build_program():
    import concourse.bass as bass
    import concourse.mybir as mybir

    FT = mybir.dt.float32
    HF = mybir.dt.float16
    BF = mybir.dt.bfloat16
    AF = mybir.ActivationFunctionType
    ALU = mybir.AluOpType

    nc = bass.Bass()
    scp_d = nc.declare_dram_parameter("scp", [N, NCOL], FT, isOutput=False)
    csth_d = nc.declare_dram_parameter("csth", [N, N + 3], FT, isOutput=False)
    cstc_d = nc.declare_dram_parameter("cstc", [N, N], FT, isOutput=False)
    ktf_d = nc.declare_dram_parameter("ktfull", [CB, SN], HF, isOutput=False)
    out_d = nc.declare_dram_parameter("out", [CB, SN], FT, isOutput=True)

    from contextlib import ExitStack

    with ExitStack() as ctx:
        es_sc = ctx.enter_context(nc.sbuf_tensor([N, NCOL], FT))
        es = ctx.enter_context(nc.sbuf_tensor([N, NCOL], FT))
        p_all = ctx.enter_context(nc.sbuf_tensor([N, NCOL], BF))
        lnp = ctx.enter_context(nc.sbuf_tensor([N, NCOL], FT))
        e_sb = ctx.enter_context(nc.sbuf_tensor([N, N], BF))
        csth = ctx.enter_context(nc.sbuf_tensor([N, N + 3], FT))
        ident = ctx.enter_context(nc.sbuf_tensor([N, N], FT))
        scr = ctx.enter_context(nc.sbuf_tensor([N, 1], FT))
        e0k = ctx.enter_context(nc.sbuf_tensor([N, 1], FT))
        ktf_sb = ctx.enter_context(nc.sbuf_tensor([CB, SN], HF))
        out_tr = ctx.enter_context(nc.sbuf_tensor([CB, SN], FT))
        drow = ctx.enter_context(nc.sbuf_tensor([1, CB], FT))
        d_sb = ctx.enter_context(nc.sbuf_tensor([CB, 1], FT))
        tr_nat = csth[0:N, 0:N]
        tcol_sb = csth[0:N, N : N + 1]
        kc_sb = csth[0:N, N + 1 : N + 3]
        s_ps = ctx.enter_context(nc.psum_tensor([N, CB], FT))
        tq0 = ctx.enter_context(nc.psum_tensor([CB, 2 * N], FT))
        tq1 = ctx.enter_context(nc.psum_tensor([CB, 2 * N], FT))
        tq2 = ctx.enter_context(nc.psum_tensor([CB, 2 * N], FT))
        tq3 = ctx.enter_context(nc.psum_tensor([CB, 2 * N], FT))
        tq9 = ctx.enter_context(nc.psum_tensor([CB, N], FT))
        d_ps = ctx.enter_context(nc.psum_tensor([CB, 1], FT))
        dm_h = ctx.enter_context(nc.semaphore())
        dm_s0 = ctx.enter_context(nc.semaphore())
        dm_s1 = ctx.enter_context(nc.semaphore())
        dm_c = ctx.enter_context(nc.semaphore())
        dm_kt = ctx.enter_context(nc.semaphore())
        dm_s2 = ctx.enter_context(nc.semaphore())
        dm_s3 = ctx.enter_context(nc.semaphore())
        sA = ctx.enter_context(nc.semaphore())
        dve = ctx.enter_context(nc.semaphore())
        pe = ctx.enter_context(nc.semaphore())
        tp_sem = ctx.enter_context(nc.semaphore())
        gpa = ctx.enter_context(nc.semaphore())
        st = ctx.enter_context(nc.semaphore())
        fin = ctx.enter_context(nc.semaphore())
        outd = ctx.enter_context(nc.semaphore())
        block = ctx.enter_context(nc.Block())
        tq = [tq0, tq1, tq2, tq3]

        @block.sync
        def _(sync):
            sync.dma_start(csth[:, :], csth_d[:, :]).then_inc(dm_h, 16)
            sync.dma_start(
                es_sc[:, 0:XCUM[0]], scp_d[:, 0:XCUM[0]]
            ).then_inc(dm_s0, 16)
            sync.dma_start(
                es_sc[:, XCUM[0] : XCUM[1]], scp_d[:, XCUM[0] : XCUM[1]]
            ).then_inc(dm_s1, 16)
            sync.dma_start(ident[:, :], cstc_d[:, :]).then_inc(dm_c, 16)
            sync.dma_start(ktf_sb[:, :], ktf_d[:, :]).then_inc(dm_kt, 16)
            sync.dma_start(
                es_sc[:, XCUM[1] : XCUM[2]], scp_d[:, XCUM[1] : XCUM[2]]
            ).then_inc(dm_s2, 16)
            sync.dma_start(
                es_sc[:, XCUM[2] : XCUM[3]], scp_d[:, XCUM[2] : XCUM[3]]
            ).then_inc(dm_s3, 16)
            for k in range(FPIECE):
                sync.wait_ge(fin, k + 1)
                sync.dma_start(
                    out_d[:, k * FC : (k + 1) * FC], out_tr[:, k * FC : (k + 1) * FC]
                ).then_inc(outd, 16)

        def tp_op(tensor, rp):
            # tq bank h%4 is freed by GPSIMD's tqa_op(h-4): gate on gpa.
            h = rp // 2
            dst = tq9[:, :] if rp == S - 1 else tq[h % 4][
                :, (rp % 2) * N : (rp % 2 + 1) * N
            ]
            if rp % 2 == 0 and 4 <= h < NPAIR:
                tensor.wait_ge(gpa, h - 3)
            t = tensor.transpose(dst, lnp[:, rp * CB : (rp + 1) * CB], ident[:, :])
            t._wait_ge(sA, 7 + h)
            t.then_inc(tp_sem, 1)

        @block.tensor
        def _(tensor):
            tensor.wait_ge(sA, 3)
            for r in range(1, S):
                mm = tensor.matmul(
                    s_ps[:, :], e_sb[:, :], p_all[:, (r - 1) * CB : r * CB]
                )
                mm._wait_ge(dve, r)
                mm.then_inc(pe, 1)
                if r == 4:
                    tensor.wait_ge(dm_c, 16)
                if r >= 4:
                    tp_op(tensor, r - 4)
            for rp in range(S - 4, S):
                tp_op(tensor, rp)
            tv = tensor.transpose(d_ps[:, :], drow[0:1, :], ident[0:1, 0:1])
            tv._wait_ge(st, 4)
            tv.then_inc(st, 1)

        @block.scalar
        def _(scalar):
            # dummy exp: pull the ACT table load into the runtime-init window
            scalar.activation(scr[:, :], scr[:, :], AF.Exp)
            scalar.wait_ge(dm_h, 16)
            scalar.activation(
                e0k[:, :], tcol_sb[:, :], AF.Exp, bias=kc_sb[:, 0:1]
            ).then_inc(sA, 1)
            scalar.wait_ge(dm_s0, 16)
            scalar.activation(
                es[:, 0:CB], es_sc[:, 0:CB], AF.Exp, bias=kc_sb[:, 1:2]
            ).then_inc(sA, 1)
            scalar.activation(e_sb[:, :], tr_nat[:, :], AF.Exp).then_inc(sA, 1)
            for k, gate in ((1, dm_s1), (2, dm_s2), (3, dm_s3)):
                scalar.wait_ge(gate, 16)
                scalar.activation(
                    es[:, XCUM[k - 1] : XCUM[k]],
                    es_sc[:, XCUM[k - 1] : XCUM[k]],
                    AF.Exp,
                    bias=kc_sb[:, 1:2],
                ).then_inc(sA, 1)
            for h in range(NPAIR):
                a = scalar.activation(
                    lnp[:, 2 * h * CB : (2 * h + 2) * CB],
                    p_all[:, 2 * h * CB : (2 * h + 2) * CB],
                    AF.Ln,
                )
                a._wait_ge(dve, 2 * h + 2)
                a.then_inc(sA, 1)
            a = scalar.activation(
                lnp[:, (S - 1) * CB : S * CB],
                p_all[:, (S - 1) * CB : S * CB],
                AF.Ln,
            )
            a._wait_ge(dve, S)
            a.then_inc(sA, 1)
            cp = scalar.copy(d_sb[:, :], d_ps[:, :])
            cp._wait_ge(st, 5)
            cp.then_inc(st, 1)

        def tqa_op(vector, h):
            # PSUM->SBUF move fused with the K*t fp16 constant add
            if h == NPAIR:
                a = vector.tensor_add(
                    out_tr[:, h * 2 * N : h * 2 * N + N],
                    tq9[:, :],
                    ktf_sb[:, h * 2 * N : h * 2 * N + N],
                )
                a._wait_ge(tp_sem, S)
            else:
                a = vector.tensor_add(
                    out_tr[:, h * 2 * N : (h + 1) * 2 * N],
                    tq[h % 4][:, :],
                    ktf_sb[:, h * 2 * N : (h + 1) * 2 * N],
                )
                a._wait_ge(tp_sem, 2 * h + 2)
            a.then_inc(gpa, 1)

        @block.vector
        def _(vector):
            vector.memset(drow[0:1, 0:BS], 0.0)
            vector.wait_ge(sA, 2)
            vector.tensor_scalar_mul(
                p_all[:, 0:CB], es[:, 0:CB], e0k[:, :]
            ).then_inc(dve, 1)
            for r in range(1, S):
                if _sa_gate(r) > _sa_gate(r - 1):
                    vector.wait_ge(sA, _sa_gate(r))
                m = vector.tensor_mul(
                    p_all[:, r * CB : (r + 1) * CB],
                    s_ps[:, :],
                    es[:, r * CB : (r + 1) * CB],
                )
                m._wait_ge(pe, r)
                m.then_inc(dve, 1)
                if r == 9:
                    vector.wait_ge(dm_kt, 16)
                if r >= 9 and (r - 9) % 2 == 0:
                    tqa_op(vector, (r - 9) // 2)
            for h in range((S - 10) // 2 + 1, NPAIR + 1):
                tqa_op(vector, h)
            # START column: alpha[...,0] carries a -10000 from transitions[:,0]
            vector.tensor_scalar_add(
                out_tr[:, 0:SN:N], out_tr[:, 0:SN:N], -10000.0
            )
            # stitch: delta[c,b] = prefix_c(lnp[S-1,0,(c-1),b] - lnp[W-1,0,c,b])
            for b in range(BS):
                sc0 = (S - 1) * CB + b
                sc1 = (W - 1) * CB + BS + b
                t = vector.tensor_tensor_scan(
                    drow[0:1, BS + b : CB : BS],
                    lnp[0:1, sc0 : sc0 + (C - 1) * BS : BS],
                    lnp[0:1, sc1 : sc1 + (C - 1) * BS : BS],
                    0.0,
                    ALU.add,
                    ALU.subtract,
                )
                if b == 0:
                    t._wait_ge(sA, 7 + NPAIR)
                t.then_inc(st, 1)
            for k in range(FPIECE):
                f = vector.tensor_scalar_add(
                    out_tr[:, k * FC : (k + 1) * FC],
                    out_tr[:, k * FC : (k + 1) * FC],
                    d_sb[:, :],
                )
                if k == 0:
                    f._wait_ge(st, 6)
                f.then_inc(fin, 1)

    return nc


LAST_RESULT = None


def kernel(scores: np.ndarray, transitions: np.ndarray) -> np.ndarray:
    global LAST_RESULT
    from concourse.bass_utils import run_bass_kernel_spmd

    scores = np.ascontiguousarray(scores, dtype=np.float32)
    transitions = np.ascontiguousarray(transitions, dtype=np.float32)

    # host-side constants and layout permutes (no math on the data path)
    idx_t = np.arange(C)[None, :] * L + np.arange(S)[:, None]      # (S, C)
    csth = np.zeros((N, N + 3), np.float32)
    csth[:, 0:N] = transitions
    # E fixup baked into the table: exp(0)=1 on column 0, exp(-1e4)=0 on row 0
    csth[:, 0] = 0.0
    csth[0, 0:N] = -10000.0
    csth[:, N] = transitions[0, :]
    csth[0, N] = 0.0               # e0k[0] = exp(K)
    csth[:, N + 1] = K
    csth[:, N + 2] = -K
    cstc = np.eye(N, dtype=np.float32)
    tvals = (np.arange(C)[:, None] * L + np.arange(S)[None, :]).astype(np.float32)
    ktf = np.repeat(K * tvals[:, None, :], BS, axis=1).reshape(CB, S)
    ktfull = np.repeat(ktf[:, :, None], N, axis=2).reshape(CB, SN)
    ktfull = np.ascontiguousarray(ktfull.astype(np.float16))

    nc = _build_program()
    in_maps = []
    for g in range(NCORES):
        blk = scores[g * BS : (g + 1) * BS]                 # (BS, T, N)
        scp = np.ascontiguousarray(
            blk[:, idx_t, :].transpose(3, 1, 2, 0).reshape(N, NCOL)
        )
        in_maps.append(
            {"scp": scp, "csth": csth, "cstc": cstc, "ktfull": ktfull}
        )
    res = run_bass_kernel_spmd(nc, in_maps, list(range(NCORES)))
    LAST_RESULT = res
    out = np.empty((B, T, N), dtype=np.float32)
    for g in range(NCORES):
        arr = res.results[g]["out"].reshape(C, BS, S, N)
        og = out[g * BS : (g + 1) * BS]
        og[:, 0:S] = arr[0]
        for c in range(1, C):
            og[:, c * L + W : c * L + S] = arr[c, :, W:S]
    return out


# revision 15
# speedup vs baseline: 1.1928x; 1.0858x over previous
"""CRF forward-algorithm kernel for Trainium2 (8 NeuronCores, Bass).

Strategy: data-parallel over batch (32 -> 4 per core) PLUS chunked-scan
parallelism over time. The recursion

    alpha_t[b,j] = scores[b,t,j] + lse_i(trans[i,j] + alpha_{t-1}[b,i])

is run in linear space with a global per-step normalizer K:

    p_t[j,(c,b)] = exp(scores - K) * sum_i E[i,j] p_{t-1}[i,(c,b)]

E = exp(trans) has entries in [e^-0.1, e^0.1], so the linear map contracts
the Hilbert projective metric by ~0.1 per step; a W=2 warmup from a
surrogate init (alpha ~ emission scores) leaves only ~1e-2 absolute error,
far inside the 2e-2-relative budget (output scale ~1e4). T=512 is split
into C=30 chunks of L=17 steps, warm-started W=2 steps early; sequential
scan length S = L+W = 19. Only a per-chunk scalar log-offset delta remains,
recovered by a DVE tensor_tensor_scan prefix over chunk-boundary
mismatches and added per-partition to the transposed output.

Pipeline: PE interleaves the scan matmuls (E stationary bf16, 120 moving
columns) with bf16 transposes of the raw state p into PSUM - gated only by
the scan itself. ACT then takes ln of the transposed PSUM directly into
the output buffer; the K*t - 10000*[j==0] constant (host fp16) and the
per-chunk delta are fused into one final scalar_tensor_tensor pass on DVE.
The delta stitch reads two single-partition ln rows of p (the K*t terms
cancel between matching boundary times). The E fixup (col 0 -> 1, row 0
-> 0) is baked into host constants as 0.0/-10000.0 entries so exp()
produces it directly. Host does only layout permutes, no math.
"""

import numpy as np

N = 64
T = 512
B = 32
NCORES = 8
BS = B // NCORES   # 4 batch elements per core
C = 30             # time chunks
W = 2              # warmup steps per chunk
L = (T - W) // C   # 17 real steps per chunk (chunk 0: L+W)
S = L + W          # 19 sequential scan steps
CB = C * BS        # 120 columns per scan step
NCOL = S * CB      # 2280 state columns
K = 4.66
NPAIR = S // 2     # 9 transposed pairs; step S-1 rides alone
SN = S * N         # 1216 output cols per partition
HC = N + 4         # header cols riding ahead of scp: trans|tcol|+K|-K|1.0
XCUM = [CB, 840, 1680, NCOL]   # cumulative cols per es exp op
FPIECE = 2
FC = SN // FPIECE  # 608 cols per final delta-add/DMA-out piece


def _sa_gate(r):
    # ACT-counter value needed before scan column-slice r is read.
    # sA: 1=e0k 2=es piece1 3=e_sb 4..6=es pieces 2..4
    need = (r + 1) * CB
    for i, c in enumerate(XCUM):
        if c >= need:
            return 2 if i == 0 else 3 + i
    return 6


def _build_program():
    import concourse.bass as bass
    import concourse.mybir as mybir

    FT = mybir.dt.float32
    HF = mybir.dt.float16
    BF = mybir.dt.bfloat16
    AF = mybir.ActivationFunctionType
    ALU = mybir.AluOpType

    nc = bass.Bass()
    scp_d = nc.declare_dram_parameter("scp", [N, HC + NCOL], FT, isOutput=False)
    idb_d = nc.declare_dram_parameter("identb", [N, N], BF, isOutput=False)
    ktf_d = nc.declare_dram_parameter("ktfull", [CB, SN], HF, isOutput=False)
    out_d = nc.declare_dram_parameter("out", [CB, SN], FT, isOutput=True)

    from contextlib import ExitStack

    with ExitStack() as ctx:
        hot = ctx.enter_context(nc.sbuf_tensor([N, HC + NCOL], FT))
        es = ctx.enter_context(nc.sbuf_tensor([N, NCOL], FT))
        p_all = ctx.enter_context(nc.sbuf_tensor([N, NCOL], BF))
        e_sb = ctx.enter_context(nc.sbuf_tensor([N, N], BF))
        identb = ctx.enter_context(nc.sbuf_tensor([N, N], BF))
        scr = ctx.enter_context(nc.sbuf_tensor([N, 1], FT))
        e0k = ctx.enter_context(nc.sbuf_tensor([N, 1], FT))
        ktf_sb = ctx.enter_context(nc.sbuf_tensor([CB, SN], HF))
        out_tr = ctx.enter_context(nc.sbuf_tensor([CB, SN], FT))
        rowW = ctx.enter_context(nc.sbuf_tensor([1, CB], FT))
        rowS = ctx.enter_context(nc.sbuf_tensor([1, CB], FT))
        drow = ctx.enter_context(nc.sbuf_tensor([1, CB], FT))
        d_sb = ctx.enter_context(nc.sbuf_tensor([CB, 1], FT))
        tr_nat = hot[0:N, 0:N]
        tcol_sb = hot[0:N, N : N + 1]
        kc_sb = hot[0:N, N + 1 : N + 3]
        one_sb = hot[0:1, N + 3 : N + 4]
        es_sc = hot[:, HC : HC + NCOL]
        s_ps = ctx.enter_context(nc.psum_tensor([N, CB], FT))
        tq0 = ctx.enter_context(nc.psum_tensor([CB, 2 * N], BF))
        tq1 = ctx.enter_context(nc.psum_tensor([CB, 2 * N], BF))
        tq2 = ctx.enter_context(nc.psum_tensor([CB, 2 * N], BF))
        tq3 = ctx.enter_context(nc.psum_tensor([CB, 2 * N], BF))
        tq9 = ctx.enter_context(nc.psum_tensor([CB, N], BF))
        d_ps = ctx.enter_context(nc.psum_tensor([CB, 1], FT))
        dm_h = ctx.enter_context(nc.semaphore())
        dm_s1 = ctx.enter_context(nc.semaphore())
        dm_c = ctx.enter_context(nc.semaphore())
        dm_s2 = ctx.enter_context(nc.semaphore())
        dm_s3 = ctx.enter_context(nc.semaphore())
        dm_kt = ctx.enter_context(nc.semaphore())
        sA = ctx.enter_context(nc.semaphore())
        lnc = ctx.enter_context(nc.semaphore())
        dve = ctx.enter_context(nc.semaphore())
        pe = ctx.enter_context(nc.semaphore())
        tp_sem = ctx.enter_context(nc.semaphore())
        st = ctx.enter_context(nc.semaphore())
        fin = ctx.enter_context(nc.semaphore())
        outd = ctx.enter_context(nc.semaphore())
        block = ctx.enter_context(nc.Block())
        tq = [tq0, tq1, tq2, tq3]

        @block.sync
        def _(sync):
            sync.dma_start(
                hot[:, 0 : HC + XCUM[0]], scp_d[:, 0 : HC + XCUM[0]]
            ).then_inc(dm_h, 16)
            sync.dma_start(
                hot[:, HC + XCUM[0] : HC + XCUM[1]],
                scp_d[:, HC + XCUM[0] : HC + XCUM[1]],
            ).then_inc(dm_s1, 16)
            sync.dma_start(identb[:, :], idb_d[:, :]).then_inc(dm_c, 16)
            sync.dma_start(
                hot[:, HC + XCUM[1] : HC + XCUM[2]],
                scp_d[:, HC + XCUM[1] : HC + XCUM[2]],
            ).then_inc(dm_s2, 16)
            sync.dma_start(
                hot[:, HC + XCUM[2] : HC + XCUM[3]],
                scp_d[:, HC + XCUM[2] : HC + XCUM[3]],
            ).then_inc(dm_s3, 16)
            sync.dma_start(ktf_sb[:, :], ktf_d[:, :]).then_inc(dm_kt, 16)
            for k in range(FPIECE):
                sync.wait_ge(fin, k + 1)
                sync.dma_start(
                    out_d[:, k * FC : (k + 1) * FC], out_tr[:, k * FC : (k + 1) * FC]
                ).then_inc(outd, 16)

        def tp_op(tensor, rp):
            # transpose raw bf16 state p into a PSUM bank; bank h%4 is
            # freed once ACT's ln pair h-4 has drained it: gate on lnc.
            h = rp // 2
            dst = tq9[:, :] if rp == S - 1 else tq[h % 4][
                :, (rp % 2) * N : (rp % 2 + 1) * N
            ]
            if rp % 2 == 0 and 4 <= h < NPAIR:
                tensor.wait_ge(lnc, h - 3)
            t = tensor.transpose(
                dst, p_all[:, rp * CB : (rp + 1) * CB], identb[:, :]
            )
            t.then_inc(tp_sem, 1)

        @block.tensor
        def _(tensor):
            tensor.wait_ge(sA, 3)
            for r in range(1, S):
                mm = tensor.matmul(
                    s_ps[:, :], e_sb[:, :], p_all[:, (r - 1) * CB : r * CB]
                )
                mm._wait_ge(dve, r)
                mm.then_inc(pe, 1)
                if r == 4:
                    tensor.wait_ge(dm_c, 16)
                if r >= 4:
                    tp_op(tensor, r - 4)
            for rp in range(S - 4, S):
                tp_op(tensor, rp)
            tv = tensor.transpose(d_ps[:, :], drow[0:1, :], one_sb[:, :])
            tv._wait_ge(st, 5)
            tv.then_inc(st, 1)

        def ln_op(scalar, h):
            # ln of a transposed PSUM pair straight into the output buffer
            if h == NPAIR:
                a = scalar.activation(
                    out_tr[:, h * 2 * N : h * 2 * N + N], tq9[:, :], AF.Ln
                )
                a._wait_ge(tp_sem, S)
            else:
                a = scalar.activation(
                    out_tr[:, h * 2 * N : (h + 1) * 2 * N], tq[h % 4][:, :], AF.Ln
                )
                a._wait_ge(tp_sem, 2 * h + 2)
            a.then_inc(lnc, 1)

        @block.scalar
        def _(scalar):
            # dummy exp: pull the ACT table load into the runtime-init window
            scalar.activation(scr[:, :], scr[:, :], AF.Exp)
            scalar.wait_ge(dm_h, 16)
            scalar.activation(
                e0k[:, :], tcol_sb[:, :], AF.Exp, bias=kc_sb[:, 0:1]
            ).then_inc(sA, 1)
            scalar.activation(
                es[:, 0:CB], es_sc[:, 0:CB], AF.Exp, bias=kc_sb[:, 1:2]
            ).then_inc(sA, 1)
            scalar.activation(e_sb[:, :], tr_nat[:, :], AF.Exp).then_inc(sA, 1)
            scalar.wait_ge(dm_s1, 16)
            scalar.activation(
                es[:, XCUM[0] : XCUM[1]],
                es_sc[:, XCUM[0] : XCUM[1]],
                AF.Exp,
                bias=kc_sb[:, 1:2],
            ).then_inc(sA, 1)
            # single-partition ln row of the warmup boundary for the stitch
            w = scalar.activation(
                rowW[0:1, :], p_all[0:1, (W - 1) * CB : W * CB], AF.Ln
            )
            w._wait_ge(dve, W)
            ln_op(scalar, 0)
            scalar.wait_ge(dm_s2, 16)
            scalar.activation(
                es[:, XCUM[1] : XCUM[2]],
                es_sc[:, XCUM[1] : XCUM[2]],
                AF.Exp,
                bias=kc_sb[:, 1:2],
            ).then_inc(sA, 1)
            ln_op(scalar, 1)
            ln_op(scalar, 2)
            scalar.wait_ge(dm_s3, 16)
            scalar.activation(
                es[:, XCUM[2] : XCUM[3]],
                es_sc[:, XCUM[2] : XCUM[3]],
                AF.Exp,
                bias=kc_sb[:, 1:2],
            ).then_inc(sA, 1)
            for h in range(3, NPAIR - 1):
                ln_op(scalar, h)
            s_ = scalar.activation(
                rowS[0:1, :], p_all[0:1, (S - 1) * CB : S * CB], AF.Ln
            )
            s_._wait_ge(dve, S)
            s_.then_inc(st, 1)
            ln_op(scalar, NPAIR - 1)
            ln_op(scalar, NPAIR)

        @block.vector
        def _(vector):
            vector.memset(drow[0:1, 0:BS], 0.0)
            vector.wait_ge(sA, 2)
            vector.tensor_scalar_mul(
                p_all[:, 0:CB], es[:, 0:CB], e0k[:, :]
            ).then_inc(dve, 1)
            for r in range(1, S):
                if _sa_gate(r) > _sa_gate(r - 1):
                    vector.wait_ge(sA, _sa_gate(r))
                m = vector.tensor_mul(
                    p_all[:, r * CB : (r + 1) * CB],
                    s_ps[:, :],
                    es[:, r * CB : (r + 1) * CB],
                )
                m._wait_ge(pe, r)
                m.then_inc(dve, 1)
            # stitch: delta[c,b] = prefix_c(rowS[(c-1),b] - rowW[c,b])
            for b in range(BS):
                t = vector.tensor_tensor_scan(
                    drow[0:1, BS + b : CB : BS],
                    rowS[0:1, b : b + (C - 1) * BS : BS],
                    rowW[0:1, BS + b : CB : BS],
                    0.0,
                    ALU.add,
                    ALU.subtract,
                )
                if b == 0:
                    t._wait_ge(st, 1)
                t.then_inc(st, 1)
            cp = vector.tensor_copy(d_sb[:, :], d_ps[:, :])
            cp._wait_ge(st, 6)
            vector.wait_ge(lnc, NPAIR + 1)
            vector.wait_ge(dm_kt, 16)
            for k in range(FPIECE):
                f = vector.scalar_tensor_tensor(
                    out_tr[:, k * FC : (k + 1) * FC],
                    out_tr[:, k * FC : (k + 1) * FC],
                    d_sb[:, :],
                    ktf_sb[:, k * FC : (k + 1) * FC],
                    ALU.add,
                    ALU.add,
                )
                f.then_inc(fin, 1)

    return nc


LAST_RESULT = None


def kernel(scores: np.ndarray, transitions: np.ndarray) -> np.ndarray:
    global LAST_RESULT
    import ml_dtypes
    from concourse.bass_utils import run_bass_kernel_spmd

    scores = np.ascontiguousarray(scores, dtype=np.float32)
    transitions = np.ascontiguousarray(transitions, dtype=np.float32)

    # host-side constants and layout permutes (no math on the data path)
    idx_t = np.arange(C)[None, :] * L + np.arange(S)[:, None]      # (S, C)
    hdr = np.zeros((N, HC), np.float32)
    hdr[:, 0:N] = transitions
    # E fixup baked into the table: exp(0)=1 on column 0, exp(-1e4)=0 on row 0
    hdr[:, 0] = 0.0
    hdr[0, 0:N] = -10000.0
    hdr[:, N] = transitions[0, :]
    hdr[0, N] = 0.0               # e0k[0] = exp(K)
    hdr[:, N + 1] = K
    hdr[:, N + 2] = -K
    hdr[0, N + 3] = 1.0           # identity for the 1-row stitch transpose
    identb = np.eye(N, dtype=ml_dtypes.bfloat16)
    tvals = (np.arange(C)[:, None] * L + np.arange(S)[None, :]).astype(np.float32)
    ktf = np.repeat(K * tvals[:, None, :], BS, axis=1).reshape(CB, S)
    ktfull = np.repeat(ktf[:, :, None], N, axis=2).reshape(CB, SN)
    ktfull[:, 0::N] -= 10000.0
    ktfull = np.ascontiguousarray(ktfull.astype(np.float16))

    nc = _build_program()
    in_maps = []
    for g in range(NCORES):
        blk = scores[g * BS : (g + 1) * BS]                 # (BS, T, N)
        scp = blk[:, idx_t, :].transpose(3, 1, 2, 0).reshape(N, NCOL)
        scp = np.ascontiguousarray(np.concatenate([hdr, scp], axis=1))
        in_maps.append({"scp": scp, "identb": identb, "ktfull": ktfull})
    res = run_bass_kernel_spmd(nc, in_maps, list(range(NCORES)))
    LAST_RESULT = res
    out = np.empty((B, T, N), dtype=np.float32)
    for g in range(NCORES):
        arr = res.results[g]["out"].reshape(C, BS, S, N)
        og = out[g * BS : (g + 1) * BS]
        og[:, 0:S] = arr[0]
        for c in range(1, C):
            og[:, c * L + W : c * L + S] = arr[c, :, W:S]
    return out


# revision 17
# speedup vs baseline: 1.2878x; 1.0796x over previous
"""CRF forward-algorithm kernel for Trainium2 (8 NeuronCores, Bass).

Strategy: data-parallel over batch (32 -> 4 per core) PLUS chunked-scan
parallelism over time. The recursion

    alpha_t[b,j] = scores[b,t,j] + lse_i(trans[i,j] + alpha_{t-1}[b,i])

is run in linear space with a global per-step normalizer K:

    p_t[j,(c,b)] = exp(scores - K) * sum_i E[i,j] p_{t-1}[i,(c,b)]

E = exp(trans) has entries in [e^-0.1, e^0.1], so the linear map contracts
the Hilbert projective metric by ~0.1 per step; a W=2 warmup from a
surrogate init (alpha ~ emission scores) leaves only ~1e-2 absolute error,
far inside the 2e-2-relative budget (output scale ~1e4). T=512 is split
into C=30 chunks of L=17 steps, warm-started W=2 steps early; sequential
scan length S = L+W = 19. Only a per-chunk scalar log-offset delta remains,
recovered by a DVE tensor_tensor_scan prefix over chunk-boundary
mismatches and added per-partition to the transposed output.

Pipeline: PE interleaves the scan matmuls (E stationary bf16, 120 moving
columns) with bf16 transposes of the raw state p into PSUM - gated only by
the scan itself. ACT then takes ln of the transposed PSUM directly into
the output buffer; the K*t - 10000*[j==0] constant (host fp16) and the
per-chunk delta are fused into one final scalar_tensor_tensor pass on DVE.
The delta stitch reads two single-partition ln rows of p (the K*t terms
cancel between matching boundary times). The E fixup (col 0 -> 1, row 0
-> 0) is baked into host constants as 0.0/-10000.0 entries so exp()
produces it directly. Host does only layout permutes, no math.
"""

import numpy as np

N = 64
T = 512
B = 32
NCORES = 8
BS = B // NCORES   # 4 batch elements per core
C = 30             # time chunks
W = 2              # warmup steps per chunk
L = (T - W) // C   # 17 real steps per chunk (chunk 0: L+W)
S = L + W          # 19 sequential scan steps
CB = C * BS        # 120 columns per scan step
NCOL = S * CB      # 2280 state columns
K = 4.66
NPAIR = S // 2     # 9 transposed pairs; step S-1 rides alone
SN = S * N         # 1216 output cols per partition
HC = N + 4         # header cols riding ahead of scp: trans|tcol|+K|-K|1.0
XCUM = [CB, 480, 840, 1680, NCOL]   # cumulative cols per es exp op
FPIECE = 1
FC = SN // FPIECE  # final delta-add/DMA-out in one fused piece


def _sa_gate(r):
    # ACT-counter value needed before scan column-slice r is read.
    # sA: 1=e0k 2=es piece1 3=e_sb 4..7=es pieces 2..5
    need = (r + 1) * CB
    for i, c in enumerate(XCUM):
        if c >= need:
            return 2 if i == 0 else 3 + i
    return 6


def _build_program():
    import concourse.bass as bass
    import concourse.mybir as mybir

    FT = mybir.dt.float32
    HF = mybir.dt.float16
    BF = mybir.dt.bfloat16
    AF = mybir.ActivationFunctionType
    ALU = mybir.AluOpType

    nc = bass.Bass()
    scp_d = nc.declare_dram_parameter("scp", [N, HC + NCOL], FT, isOutput=False)
    idb_d = nc.declare_dram_parameter("identb", [N, N], BF, isOutput=False)
    ktf_d = nc.declare_dram_parameter("ktfull", [CB, SN], HF, isOutput=False)
    out_d = nc.declare_dram_parameter("out", [CB, SN], FT, isOutput=True)

    from contextlib import ExitStack

    with ExitStack() as ctx:
        hot = ctx.enter_context(nc.sbuf_tensor([N, HC + NCOL], FT))
        es = ctx.enter_context(nc.sbuf_tensor([N, NCOL], FT))
        p_all = ctx.enter_context(nc.sbuf_tensor([N, NCOL], BF))
        e_sb = ctx.enter_context(nc.sbuf_tensor([N, N], BF))
        identb = ctx.enter_context(nc.sbuf_tensor([N, N], BF))
        scr = ctx.enter_context(nc.sbuf_tensor([N, 1], FT))
        e0k = ctx.enter_context(nc.sbuf_tensor([N, 1], FT))
        ktf_sb = ctx.enter_context(nc.sbuf_tensor([CB, SN], HF))
        out_tr = ctx.enter_context(nc.sbuf_tensor([CB, SN], FT))
        rowW = ctx.enter_context(nc.sbuf_tensor([1, CB], FT))
        rowS = ctx.enter_context(nc.sbuf_tensor([1, CB], FT))
        drow = ctx.enter_context(nc.sbuf_tensor([1, CB], FT))
        d_sb = ctx.enter_context(nc.sbuf_tensor([CB, 1], FT))
        tr_nat = hot[0:N, 0:N]
        tcol_sb = hot[0:N, N : N + 1]
        kc_sb = hot[0:N, N + 1 : N + 3]
        one_sb = hot[0:1, N + 3 : N + 4]
        es_sc = hot[:, HC : HC + NCOL]
        s_ps = ctx.enter_context(nc.psum_tensor([N, CB], FT))
        tq0 = ctx.enter_context(nc.psum_tensor([CB, 2 * N], BF))
        tq1 = ctx.enter_context(nc.psum_tensor([CB, 2 * N], BF))
        tq2 = ctx.enter_context(nc.psum_tensor([CB, 2 * N], BF))
        tq3 = ctx.enter_context(nc.psum_tensor([CB, 2 * N], BF))
        tq9 = ctx.enter_context(nc.psum_tensor([CB, N], BF))
        d_ps = ctx.enter_context(nc.psum_tensor([CB, 1], FT))
        dm_h = ctx.enter_context(nc.semaphore())
        dm_s1 = ctx.enter_context(nc.semaphore())
        dm_c = ctx.enter_context(nc.semaphore())
        dm_s2 = ctx.enter_context(nc.semaphore())
        dm_s3 = ctx.enter_context(nc.semaphore())
        dm_s4 = ctx.enter_context(nc.semaphore())
        dm_kt = ctx.enter_context(nc.semaphore())
        sA = ctx.enter_context(nc.semaphore())
        lnc = ctx.enter_context(nc.semaphore())
        dve = ctx.enter_context(nc.semaphore())
        pe = ctx.enter_context(nc.semaphore())
        tp_sem = ctx.enter_context(nc.semaphore())
        st = ctx.enter_context(nc.semaphore())
        fin = ctx.enter_context(nc.semaphore())
        outd = ctx.enter_context(nc.semaphore())
        block = ctx.enter_context(nc.Block())
        tq = [tq0, tq1, tq2, tq3]

        @block.sync
        def _(sync):
            sync.dma_start(
                hot[:, 0 : HC + XCUM[0]], scp_d[:, 0 : HC + XCUM[0]]
            ).then_inc(dm_h, 16)
            sync.dma_start(
                hot[:, HC + XCUM[0] : HC + XCUM[1]],
                scp_d[:, HC + XCUM[0] : HC + XCUM[1]],
            ).then_inc(dm_s1, 16)
            sync.dma_start(
                hot[:, HC + XCUM[1] : HC + XCUM[2]],
                scp_d[:, HC + XCUM[1] : HC + XCUM[2]],
            ).then_inc(dm_s2, 16)
            sync.dma_start(identb[:, :], idb_d[:, :]).then_inc(dm_c, 16)
            sync.dma_start(
                hot[:, HC + XCUM[2] : HC + XCUM[3]],
                scp_d[:, HC + XCUM[2] : HC + XCUM[3]],
            ).then_inc(dm_s3, 16)
            sync.dma_start(
                hot[:, HC + XCUM[3] : HC + XCUM[4]],
                scp_d[:, HC + XCUM[3] : HC + XCUM[4]],
            ).then_inc(dm_s4, 16)
            sync.dma_start(ktf_sb[:, :], ktf_d[:, :]).then_inc(dm_kt, 16)
            for k in range(FPIECE):
                sync.wait_ge(fin, k + 1)
                sync.dma_start(
                    out_d[:, k * FC : (k + 1) * FC], out_tr[:, k * FC : (k + 1) * FC]
                ).then_inc(outd, 16)

        def tp_op(tensor, rp):
            # transpose raw bf16 state p into a PSUM bank; bank h%4 is
            # freed once ACT's ln pair h-4 has drained it: gate on lnc.
            h = rp // 2
            dst = tq9[:, :] if rp == S - 1 else tq[h % 4][
                :, (rp % 2) * N : (rp % 2 + 1) * N
            ]
            if rp % 2 == 0 and 4 <= h < NPAIR:
                tensor.wait_ge(lnc, h - 3)
            t = tensor.transpose(
                dst, p_all[:, rp * CB : (rp + 1) * CB], identb[:, :]
            )
            t.then_inc(tp_sem, 1)

        @block.tensor
        def _(tensor):
            tensor.wait_ge(sA, 3)
            for r in range(1, S):
                mm = tensor.matmul(
                    s_ps[:, :], e_sb[:, :], p_all[:, (r - 1) * CB : r * CB]
                )
                mm._wait_ge(dve, r)
                mm.then_inc(pe, 1)
                if r == 1:
                    tensor.wait_ge(dm_c, 16)
                tp_op(tensor, r - 1)
            tensor.wait_ge(dve, S)
            tp_op(tensor, S - 1)
            tv = tensor.transpose(d_ps[:, :], drow[0:1, :], one_sb[:, :])
            tv._wait_ge(st, 5)
            tv.then_inc(st, 1)

        def ln_op(scalar, h):
            # ln of a transposed PSUM pair straight into the output buffer
            if h == NPAIR:
                a = scalar.activation(
                    out_tr[:, h * 2 * N : h * 2 * N + N], tq9[:, :], AF.Ln
                )
                a._wait_ge(tp_sem, S)
            else:
                a = scalar.activation(
                    out_tr[:, h * 2 * N : (h + 1) * 2 * N], tq[h % 4][:, :], AF.Ln
                )
                a._wait_ge(tp_sem, 2 * h + 2)
            a.then_inc(lnc, 1)

        @block.scalar
        def _(scalar):
            # dummy exp: pull the ACT table load into the runtime-init window
            scalar.activation(scr[:, :], scr[:, :], AF.Exp)
            scalar.wait_ge(dm_h, 16)
            scalar.activation(
                e0k[:, :], tcol_sb[:, :], AF.Exp, bias=kc_sb[:, 0:1]
            ).then_inc(sA, 1)
            scalar.activation(
                es[:, 0:CB], es_sc[:, 0:CB], AF.Exp, bias=kc_sb[:, 1:2]
            ).then_inc(sA, 1)
            scalar.activation(e_sb[:, :], tr_nat[:, :], AF.Exp).then_inc(sA, 1)
            scalar.wait_ge(dm_s1, 16)
            scalar.activation(
                es[:, XCUM[0] : XCUM[1]],
                es_sc[:, XCUM[0] : XCUM[1]],
                AF.Exp,
                bias=kc_sb[:, 1:2],
            ).then_inc(sA, 1)
            scalar.wait_ge(dm_s2, 16)
            scalar.activation(
                es[:, XCUM[1] : XCUM[2]],
                es_sc[:, XCUM[1] : XCUM[2]],
                AF.Exp,
                bias=kc_sb[:, 1:2],
            ).then_inc(sA, 1)
            # single-partition ln row of the warmup boundary for the stitch
            w = scalar.activation(
                rowW[0:1, :], p_all[0:1, (W - 1) * CB : W * CB], AF.Ln
            )
            w._wait_ge(dve, W)
            ln_op(scalar, 0)
            scalar.wait_ge(dm_s3, 16)
            scalar.activation(
                es[:, XCUM[2] : XCUM[3]],
                es_sc[:, XCUM[2] : XCUM[3]],
                AF.Exp,
                bias=kc_sb[:, 1:2],
            ).then_inc(sA, 1)
            ln_op(scalar, 1)
            ln_op(scalar, 2)
            scalar.wait_ge(dm_s4, 16)
            scalar.activation(
                es[:, XCUM[3] : XCUM[4]],
                es_sc[:, XCUM[3] : XCUM[4]],
                AF.Exp,
                bias=kc_sb[:, 1:2],
            ).then_inc(sA, 1)
            for h in range(3, NPAIR - 1):
                ln_op(scalar, h)
            s_ = scalar.activation(
                rowS[0:1, :], p_all[0:1, (S - 1) * CB : S * CB], AF.Ln
            )
            s_._wait_ge(dve, S)
            s_.then_inc(st, 1)
            ln_op(scalar, NPAIR - 1)
            ln_op(scalar, NPAIR)

        @block.vector
        def _(vector):
            vector.memset(drow[0:1, 0:BS], 0.0)
            vector.wait_ge(sA, 2)
            vector.tensor_scalar_mul(
                p_all[:, 0:CB], es[:, 0:CB], e0k[:, :]
            ).then_inc(dve, 1)
            for r in range(1, S):
                if _sa_gate(r) > _sa_gate(r - 1):
                    vector.wait_ge(sA, _sa_gate(r))
                m = vector.tensor_mul(
                    p_all[:, r * CB : (r + 1) * CB],
                    s_ps[:, :],
                    es[:, r * CB : (r + 1) * CB],
                )
                m._wait_ge(pe, r)
                m.then_inc(dve, 1)
            # stitch: delta[c,b] = prefix_c(rowS[(c-1),b] - rowW[c,b])
            for b in range(BS):
                t = vector.tensor_tensor_scan(
                    drow[0:1, BS + b : CB : BS],
                    rowS[0:1, b : b + (C - 1) * BS : BS],
                    rowW[0:1, BS + b : CB : BS],
                    0.0,
                    ALU.add,
                    ALU.subtract,
                )
                if b == 0:
                    t._wait_ge(st, 1)
                t.then_inc(st, 1)
            cp = vector.tensor_copy(d_sb[:, :], d_ps[:, :])
            cp._wait_ge(st, 6)
            vector.wait_ge(lnc, NPAIR + 1)
            vector.wait_ge(dm_kt, 16)
            for k in range(FPIECE):
                f = vector.scalar_tensor_tensor(
                    out_tr[:, k * FC : (k + 1) * FC],
                    out_tr[:, k * FC : (k + 1) * FC],
                    d_sb[:, :],
                    ktf_sb[:, k * FC : (k + 1) * FC],
                    ALU.add,
                    ALU.add,
                )
                f.then_inc(fin, 1)

    return nc


LAST_RESULT = None


def kernel(scores: np.ndarray, transitions: np.ndarray) -> np.ndarray:
    global LAST_RESULT
    import ml_dtypes
    from concourse.bass_utils import run_bass_kernel_spmd

    scores = np.ascontiguousarray(scores, dtype=np.float32)
    transitions = np.ascontiguousarray(transitions, dtype=np.float32)

    # host-side constants and layout permutes (no math on the data path)
    idx_t = np.arange(C)[None, :] * L + np.arange(S)[:, None]      # (S, C)
    hdr = np.zeros((N, HC), np.float32)
    hdr[:, 0:N] = transitions
    # E fixup baked into the table: exp(0)=1 on column 0, exp(-1e4)=0 on row 0
    hdr[:, 0] = 0.0
    hdr[0, 0:N] = -10000.0
    hdr[:, N] = transitions[0, :]
    hdr[0, N] = 0.0               # e0k[0] = exp(K)
    hdr[:, N + 1] = K
    hdr[:, N + 2] = -K
    hdr[0, N + 3] = 1.0           # identity for the 1-row stitch transpose
    identb = np.eye(N, dtype=ml_dtypes.bfloat16)
    tvals = (np.arange(C)[:, None] * L + np.arange(S)[None, :]).astype(np.float32)
    ktf = np.repeat(K * tvals[:, None, :], BS, axis=1).reshape(CB, S)
    ktfull = np.repeat(ktf[:, :, None], N, axis=2).reshape(CB, SN)
    ktfull[:, 0::N] -= 10000.0
    ktfull = np.ascontiguousarray(ktfull.astype(np.float16))

    nc = _build_program()
    in_maps = []
    for g in range(NCORES):
        blk = scores[g * BS : (g + 1) * BS]                 # (BS, T, N)
        scp = blk[:, idx_t, :].transpose(3, 1, 2, 0).reshape(N, NCOL)
        scp = np.ascontiguousarray(np.concatenate([hdr, scp], axis=1))
        in_maps.append({"scp": scp, "identb": identb, "ktfull": ktfull})
    res = run_bass_kernel_spmd(nc, in_maps, list(range(NCORES)))
    LAST_RESULT = res
    out = np.empty((B, T, N), dtype=np.float32)
    for g in range(NCORES):
        arr = res.results[g]["out"].reshape(C, BS, S, N)
        og = out[g * BS : (g + 1) * BS]
        og[:, 0:S] = arr[0]
        for c in range(1, C):
            og[:, c * L + W : c * L + S] = arr[c, :, W:S]
    return out


# revision 18
# speedup vs baseline: 1.3131x; 1.0197x over previous
"""CRF forward-algorithm kernel for Trainium2 (8 NeuronCores, Bass).

Strategy: data-parallel over batch (32 -> 4 per core) PLUS chunked-scan
parallelism over time. The recursion

    alpha_t[b,j] = scores[b,t,j] + lse_i(trans[i,j] + alpha_{t-1}[b,i])

is run in linear space with a global per-step normalizer K:

    p_t[j,(c,b)] = exp(scores - K) * sum_i E[i,j] p_{t-1}[i,(c,b)]

E = exp(trans) has entries in [e^-0.1, e^0.1], so the linear map contracts
the Hilbert projective metric by ~0.1 per step; a W=2 warmup from a
surrogate init (alpha ~ emission scores) leaves only ~1e-2 absolute error,
far inside the 2e-2-relative budget (output scale ~1e4). T=512 is split
into C=30 chunks of L=17 steps, warm-started W=2 steps early; sequential
scan length S = L+W = 19. Only a per-chunk scalar log-offset delta remains,
recovered by a DVE tensor_tensor_scan prefix over chunk-boundary
mismatches and added per-partition to the transposed output.

Pipeline: PE interleaves the scan matmuls (E stationary bf16, 120 moving
columns) with bf16 transposes of the raw state p into PSUM - gated only by
the scan itself. ACT then takes ln of the transposed PSUM directly into
the output buffer; the K*t - 10000*[j==0] constant (host fp16) and the
per-chunk delta are fused into one final scalar_tensor_tensor pass on DVE.
The delta stitch reads two single-partition ln rows of p (the K*t terms
cancel between matching boundary times). The E fixup (col 0 -> 1, row 0
-> 0) is baked into host constants as 0.0/-10000.0 entries so exp()
produces it directly. Host does only layout permutes, no math.
"""

import numpy as np

N = 64
T = 512
B = 32
NCORES = 8
BS = B // NCORES   # 4 batch elements per core
C = 30             # time chunks
W = 2              # warmup steps per chunk
L = (T - W) // C   # 17 real steps per chunk (chunk 0: L+W)
S = L + W          # 19 sequential scan steps
CB = C * BS        # 120 columns per scan step
NCOL = S * CB      # 2280 state columns
K = 4.66
NPAIR = S // 2     # 9 transposed pairs; step S-1 rides alone
SN = S * N         # 1216 output cols per partition
HC = N + 4         # header cols riding ahead of scp: trans|tcol|+K|-K|1.0
XCUM = [CB, 480, 840, 1680, NCOL]   # cumulative cols per es exp op
FPIECE = 1
FC = SN // FPIECE  # final delta-add/DMA-out in one fused piece


def _sa_gate(r):
    # ACT-counter value needed before scan column-slice r is read.
    # sA: 1=e0k 2=es piece1 3=e_sb 4..7=es pieces 2..5
    need = (r + 1) * CB
    for i, c in enumerate(XCUM):
        if c >= need:
            return 2 if i == 0 else 3 + i
    return 6


def _build_program():
    import concourse.bass as bass
    import concourse.mybir as mybir

    FT = mybir.dt.float32
    HF = mybir.dt.float16
    BF = mybir.dt.bfloat16
    AF = mybir.ActivationFunctionType
    ALU = mybir.AluOpType

    nc = bass.Bass()
    scp_d = nc.declare_dram_parameter("scp", [N, HC + NCOL], FT, isOutput=False)
    idb_d = nc.declare_dram_parameter("identb", [N, N], BF, isOutput=False)
    ktf_d = nc.declare_dram_parameter("ktfull", [CB, SN], HF, isOutput=False)
    out_d = nc.declare_dram_parameter("out", [CB, SN], FT, isOutput=True)

    from contextlib import ExitStack

    with ExitStack() as ctx:
        hot = ctx.enter_context(nc.sbuf_tensor([N, HC + NCOL], FT))
        es = ctx.enter_context(nc.sbuf_tensor([N, NCOL], FT))
        p_all = ctx.enter_context(nc.sbuf_tensor([N, NCOL], BF))
        e_sb = ctx.enter_context(nc.sbuf_tensor([N, N], BF))
        identb = ctx.enter_context(nc.sbuf_tensor([N, N], BF))
        scr = ctx.enter_context(nc.sbuf_tensor([N, 1], FT))
        e0k = ctx.enter_context(nc.sbuf_tensor([N, 1], FT))
        ktf_sb = ctx.enter_context(nc.sbuf_tensor([CB, SN], HF))
        out_tr = ctx.enter_context(nc.sbuf_tensor([CB, SN], FT))
        rowW = ctx.enter_context(nc.sbuf_tensor([1, CB], FT))
        rowS = ctx.enter_context(nc.sbuf_tensor([1, CB], FT))
        drow = ctx.enter_context(nc.sbuf_tensor([1, CB], FT))
        d_sb = ctx.enter_context(nc.sbuf_tensor([CB, 1], FT))
        tr_nat = hot[0:N, 0:N]
        tcol_sb = hot[0:N, N : N + 1]
        kc_sb = hot[0:N, N + 1 : N + 3]
        one_sb = hot[0:1, N + 3 : N + 4]
        es_sc = hot[:, HC : HC + NCOL]
        s_ps = ctx.enter_context(nc.psum_tensor([N, CB], FT))
        tq0 = ctx.enter_context(nc.psum_tensor([CB, 2 * N], BF))
        tq1 = ctx.enter_context(nc.psum_tensor([CB, 2 * N], BF))
        tq2 = ctx.enter_context(nc.psum_tensor([CB, 2 * N], BF))
        tq3 = ctx.enter_context(nc.psum_tensor([CB, 2 * N], BF))
        tq9 = ctx.enter_context(nc.psum_tensor([CB, N], BF))
        d_ps = ctx.enter_context(nc.psum_tensor([CB, 1], FT))
        dm_h = ctx.enter_context(nc.semaphore())
        dm_s1 = ctx.enter_context(nc.semaphore())
        dm_c = ctx.enter_context(nc.semaphore())
        dm_s2 = ctx.enter_context(nc.semaphore())
        dm_s3 = ctx.enter_context(nc.semaphore())
        dm_s4 = ctx.enter_context(nc.semaphore())
        dm_kt = ctx.enter_context(nc.semaphore())
        sA = ctx.enter_context(nc.semaphore())
        lnc = ctx.enter_context(nc.semaphore())
        dve = ctx.enter_context(nc.semaphore())
        pe = ctx.enter_context(nc.semaphore())
        tp_sem = ctx.enter_context(nc.semaphore())
        st = ctx.enter_context(nc.semaphore())
        fin = ctx.enter_context(nc.semaphore())
        outd = ctx.enter_context(nc.semaphore())
        block = ctx.enter_context(nc.Block())
        tq = [tq0, tq1, tq2, tq3]

        @block.sync
        def _(sync):
            sync.dma_start(
                hot[:, 0 : HC + XCUM[0]], scp_d[:, 0 : HC + XCUM[0]]
            ).then_inc(dm_h, 16)
            sync.dma_start(
                hot[:, HC + XCUM[0] : HC + XCUM[1]],
                scp_d[:, HC + XCUM[0] : HC + XCUM[1]],
            ).then_inc(dm_s1, 16)
            sync.dma_start(
                hot[:, HC + XCUM[1] : HC + XCUM[2]],
                scp_d[:, HC + XCUM[1] : HC + XCUM[2]],
            ).then_inc(dm_s2, 16)
            sync.dma_start(identb[:, :], idb_d[:, :]).then_inc(dm_c, 16)
            sync.dma_start(
                hot[:, HC + XCUM[2] : HC + XCUM[3]],
                scp_d[:, HC + XCUM[2] : HC + XCUM[3]],
            ).then_inc(dm_s3, 16)
            sync.dma_start(
                hot[:, HC + XCUM[3] : HC + XCUM[4]],
                scp_d[:, HC + XCUM[3] : HC + XCUM[4]],
            ).then_inc(dm_s4, 16)
            sync.dma_start(ktf_sb[:, :], ktf_d[:, :]).then_inc(dm_kt, 16)
            for k in range(FPIECE):
                sync.wait_ge(fin, k + 1)
                sync.dma_start(
                    out_d[:, k * FC : (k + 1) * FC], out_tr[:, k * FC : (k + 1) * FC]
                ).then_inc(outd, 16)

        def tp_op(tensor, rp):
            # transpose raw bf16 state p into a PSUM bank; bank h%4 is
            # freed once ACT's ln pair h-4 has drained it: gate on lnc.
            h = rp // 2
            dst = tq9[:, :] if rp == S - 1 else tq[h % 4][
                :, (rp % 2) * N : (rp % 2 + 1) * N
            ]
            if rp % 2 == 0 and 4 <= h < NPAIR:
                tensor.wait_ge(lnc, h - 3)
            t = tensor.transpose(
                dst, p_all[:, rp * CB : (rp + 1) * CB], identb[:, :]
            )
            t.then_inc(tp_sem, 1)

        @block.tensor
        def _(tensor):
            tensor.wait_ge(sA, 3)
            for r in range(1, S):
                mm = tensor.matmul(
                    s_ps[:, :], e_sb[:, :], p_all[:, (r - 1) * CB : r * CB]
                )
                mm._wait_ge(dve, r)
                mm.then_inc(pe, 1)
                if r == 1:
                    tensor.wait_ge(dm_c, 16)
                tp_op(tensor, r - 1)
            tensor.wait_ge(dve, S)
            tp_op(tensor, S - 1)
            tv = tensor.transpose(d_ps[:, :], drow[0:1, :], one_sb[:, :])
            tv._wait_ge(st, 5)
            tv.then_inc(st, 1)

        def ln_op(scalar, h):
            # ln of a transposed PSUM pair straight into the output buffer
            if h == NPAIR:
                a = scalar.activation(
                    out_tr[:, h * 2 * N : h * 2 * N + N], tq9[:, :], AF.Ln
                )
                a._wait_ge(tp_sem, S)
            else:
                a = scalar.activation(
                    out_tr[:, h * 2 * N : (h + 1) * 2 * N], tq[h % 4][:, :], AF.Ln
                )
                a._wait_ge(tp_sem, 2 * h + 2)
            a.then_inc(lnc, 1)

        @block.scalar
        def _(scalar):
            # dummy exp: pull the ACT table load into the runtime-init window
            scalar.activation(scr[:, :], scr[:, :], AF.Exp)
            scalar.wait_ge(dm_h, 16)
            scalar.activation(
                e0k[:, :], tcol_sb[:, :], AF.Exp, bias=kc_sb[:, 0:1]
            ).then_inc(sA, 1)
            scalar.activation(
                es[:, 0:CB], es_sc[:, 0:CB], AF.Exp, bias=kc_sb[:, 1:2]
            ).then_inc(sA, 1)
            scalar.activation(e_sb[:, :], tr_nat[:, :], AF.Exp).then_inc(sA, 1)
            scalar.wait_ge(dm_s1, 16)
            scalar.activation(
                es[:, XCUM[0] : XCUM[1]],
                es_sc[:, XCUM[0] : XCUM[1]],
                AF.Exp,
                bias=kc_sb[:, 1:2],
            ).then_inc(sA, 1)
            scalar.wait_ge(dm_s2, 16)
            scalar.activation(
                es[:, XCUM[1] : XCUM[2]],
                es_sc[:, XCUM[1] : XCUM[2]],
                AF.Exp,
                bias=kc_sb[:, 1:2],
            ).then_inc(sA, 1)
            # single-partition ln rows for the stitch: boundary time
            # (c-1)*L + (L) == c*L + 0, i.e. chunk c-1 step L vs chunk c init
            w = scalar.activation(rowW[0:1, :], p_all[0:1, 0:CB], AF.Ln)
            w._wait_ge(dve, 1)
            ln_op(scalar, 0)
            scalar.wait_ge(dm_s3, 16)
            scalar.activation(
                es[:, XCUM[2] : XCUM[3]],
                es_sc[:, XCUM[2] : XCUM[3]],
                AF.Exp,
                bias=kc_sb[:, 1:2],
            ).then_inc(sA, 1)
            ln_op(scalar, 1)
            ln_op(scalar, 2)
            scalar.wait_ge(dm_s4, 16)
            scalar.activation(
                es[:, XCUM[3] : XCUM[4]],
                es_sc[:, XCUM[3] : XCUM[4]],
                AF.Exp,
                bias=kc_sb[:, 1:2],
            ).then_inc(sA, 1)
            for h in range(3, NPAIR - 1):
                ln_op(scalar, h)
            s_ = scalar.activation(
                rowS[0:1, :], p_all[0:1, L * CB : (L + 1) * CB], AF.Ln
            )
            s_._wait_ge(dve, L + 1)
            s_.then_inc(st, 1)
            ln_op(scalar, NPAIR - 1)
            ln_op(scalar, NPAIR)

        @block.vector
        def _(vector):
            vector.memset(drow[0:1, 0:BS], 0.0)
            vector.wait_ge(sA, 2)
            vector.tensor_scalar_mul(
                p_all[:, 0:CB], es[:, 0:CB], e0k[:, :]
            ).then_inc(dve, 1)
            for r in range(1, S):
                if _sa_gate(r) > _sa_gate(r - 1):
                    vector.wait_ge(sA, _sa_gate(r))
                m = vector.tensor_mul(
                    p_all[:, r * CB : (r + 1) * CB],
                    s_ps[:, :],
                    es[:, r * CB : (r + 1) * CB],
                )
                m._wait_ge(pe, r)
                m.then_inc(dve, 1)
            # stitch: delta[c,b] = prefix_c(rowS[(c-1),b] - rowW[c,b])
            for b in range(BS):
                t = vector.tensor_tensor_scan(
                    drow[0:1, BS + b : CB : BS],
                    rowS[0:1, b : b + (C - 1) * BS : BS],
                    rowW[0:1, BS + b : CB : BS],
                    0.0,
                    ALU.add,
                    ALU.subtract,
                )
                if b == 0:
                    t._wait_ge(st, 1)
                t.then_inc(st, 1)
            cp = vector.tensor_copy(d_sb[:, :], d_ps[:, :])
            cp._wait_ge(st, 6)
            vector.wait_ge(lnc, NPAIR + 1)
            vector.wait_ge(dm_kt, 16)
            for k in range(FPIECE):
                f = vector.scalar_tensor_tensor(
                    out_tr[:, k * FC : (k + 1) * FC],
                    out_tr[:, k * FC : (k + 1) * FC],
                    d_sb[:, :],
                    ktf_sb[:, k * FC : (k + 1) * FC],
                    ALU.add,
                    ALU.add,
                )
                f.then_inc(fin, 1)

    return nc


LAST_RESULT = None


def kernel(scores: np.ndarray, transitions: np.ndarray) -> np.ndarray:
    global LAST_RESULT
    import ml_dtypes
    from concourse.bass_utils import run_bass_kernel_spmd

    scores = np.ascontiguousarray(scores, dtype=np.float32)
    transitions = np.ascontiguousarray(transitions, dtype=np.float32)

    # host-side constants and layout permutes (no math on the data path)
    idx_t = np.arange(C)[None, :] * L + np.arange(S)[:, None]      # (S, C)
    hdr = np.zeros((N, HC), np.float32)
    hdr[:, 0:N] = transitions
    # E fixup baked into the table: exp(0)=1 on column 0, exp(-1e4)=0 on row 0
    hdr[:, 0] = 0.0
    hdr[0, 0:N] = -10000.0
    hdr[:, N] = transitions[0, :]
    hdr[0, N] = 0.0               # e0k[0] = exp(K)
    hdr[:, N + 1] = K
    hdr[:, N + 2] = -K
    hdr[0, N + 3] = 1.0           # identity for the 1-row stitch transpose
    identb = np.eye(N, dtype=ml_dtypes.bfloat16)
    tvals = (np.arange(C)[:, None] * L + np.arange(S)[None, :]).astype(np.float32)
    ktf = np.repeat(K * tvals[:, None, :], BS, axis=1).reshape(CB, S)
    ktfull = np.repeat(ktf[:, :, None], N, axis=2).reshape(CB, SN)
    ktfull[:, 0::N] -= 10000.0
    ktfull = np.ascontiguousarray(ktfull.astype(np.float16))

    nc = _build_program()
    in_maps = []
    for g in range(NCORES):
        blk = scores[g * BS : (g + 1) * BS]                 # (BS, T, N)
        scp = blk[:, idx_t, :].transpose(3, 1, 2, 0).reshape(N, NCOL)
        scp = np.ascontiguousarray(np.concatenate([hdr, scp], axis=1))
        in_maps.append({"scp": scp, "identb": identb, "ktfull": ktfull})
    res = run_bass_kernel_spmd(nc, in_maps, list(range(NCORES)))
    LAST_RESULT = res
    out = np.empty((B, T, N), dtype=np.float32)
    for g in range(NCORES):
        arr = res.results[g]["out"].reshape(C, BS, S, N)
        og = out[g * BS : (g + 1) * BS]
        og[:, 0:S] = arr[0]
        for c in range(1, C):
            og[:, c * L + W : c * L + S] = arr[c, :, W:S]
    return out
